# revision 23
# baseline (speedup 1.0000x reference)
"""GPT-Neo (6-layer, hidden 1024, seq 2048) forward pass on 8 TRN2 NeuronCores.

Sharding: sequence-parallel transformer (256 tokens/core) with per-layer
AllGather of K/V; attention in transposed-score orientation with max-free
softmax and additive causal/window masks fed as per-core data; vocab-sharded
tied-lm-head GEMM at the end (logits computed transposed, [vocab_shard, 2048]
per core, unsharded + f32-cast on host).

Numerics: fp16 operands for projection/MLP/logits GEMMs (fp32 PSUM), f32
residual stream, exp/attn-weights and V in bf16 for 1-cycle/row context
matmuls, softmax denominators + reciprocals in f32.

DMA: weight streams batched into [128, 2048]-shaped tiles via rearranged
access patterns and issued on the Pool engine (SWDGE) to keep the HWDGE
queue free for activation traffic; K/V/mask/logit IO batched similarly.
"""
import sys
import numpy as np

sys.path.insert(0, "/opt/trn_rl_repo")

import concourse.bass as bass
import concourse.tile as tile
from concourse import mybir, bacc
from concourse.bass_utils import run_bass_kernel_spmd
from concourse.masks import make_identity

NCORES = 8
T = 2048
TL = T // NCORES   # 256 tokens per core
H = 1024
HEADS = 16
HD = 64
MLP = 4096
NL = 6
WINDOW = 256
VOCAB = 50257
VSH = 6400         # padded per-core vocab shard (8*6400 = 51200)
EPS = 1e-5
ATTN_LOCAL = [False, True, False, True, False, True]

F16 = mybir.dt.float16
F32 = mybir.dt.float32
BF16 = mybir.dt.bfloat16

KB = T // 128      # 16 key blocks
HP = HEADS // 2    # 8 head pairs
RG = [list(range(NCORES))]


def build(n_layers=NL, with_logits=True):
    nc = bacc.Bacc(num_devices=NCORES)

    x0_e = nc.declare_dram_parameter("x0", [TL, H], F32, isOutput=False)
    wq_e = nc.declare_dram_parameter("wq", [n_layers, H, H], F16, isOutput=False)
    wk_e = nc.declare_dram_parameter("wk", [n_layers, H, H], F16, isOutput=False)
    wv_e = nc.declare_dram_parameter("wv", [n_layers, H, H], F16, isOutput=False)
    wo_e = nc.declare_dram_parameter("wo", [n_layers, H, H], F16, isOutput=False)
    wf_e = nc.declare_dram_parameter("wf", [n_layers, H, MLP], F16, isOutput=False)
    wp_e = nc.declare_dram_parameter("wp", [n_layers, MLP, H], F16, isOutput=False)
    qb_e = nc.declare_dram_parameter("qb", [n_layers, 128, 8], F32, isOutput=False)
    kb_e = nc.declare_dram_parameter("kb", [n_layers, 128, 8], F32, isOutput=False)
    vb_e = nc.declare_dram_parameter("vb", [n_layers, 1, H], F16, isOutput=False)
    ob_e = nc.declare_dram_parameter("ob", [n_layers, 1, H], F16, isOutput=False)
    fb_e = nc.declare_dram_parameter("fb", [n_layers, 128, 32], F32, isOutput=False)
    pb_e = nc.declare_dram_parameter("pb", [n_layers, 1, H], F16, isOutput=False)
    mg_e = nc.declare_dram_parameter("maskg", [KB, 128, TL], BF16, isOutput=False)
    # local-attention masks in 8-slot pair-gather layout (A-buffer slots 0-3,
    # B-buffer slots 4-7); non-designated buffer fully masked per core
    ml_e = nc.declare_dram_parameter("maskl", [8, 128, TL], BF16, isOutput=False)
    if with_logits:
        lm_e = nc.declare_dram_parameter("lm", [H, VSH], F16, isOutput=False)
        lbt_e = nc.declare_dram_parameter("lbt", [128, VSH // 128], F32, isOutput=False)
        out_e = nc.declare_dram_parameter("out", [VSH, T], F16, isOutput=True)
    else:
        out_e = nc.declare_dram_parameter("out", [TL, H], F32, isOutput=True)

    from contextlib import ExitStack
    with tile.TileContext(nc) as tc:
        with ExitStack() as _stk:
            _p = lambda *a, **kw: _stk.enter_context(tc.tile_pool(*a, **kw))
            constp = _p(name="const", bufs=1)
            xresp = _p(name="xres", bufs=3)     # [128,1024] f32 residual
            hpoolp = _p(name="hpool", bufs=3)   # [128,1024] f16 ln out
            smallp = _p(name="small", bufs=3)
            ps_sc = _p(name="ps_sc", bufs=3, space="PSUM")
            ps_ctx = _p(name="ps_ctx", bufs=2, space="PSUM")
            ps_mm = _p(name="ps_mm", bufs=2, space="PSUM")
            dramp = _p(name="dram", bufs=2, space="DRAM")

            ident = constp.tile([128, 128], F16, name="ident")
            make_identity(nc, ident[:])
            ones_col = constp.tile([128, 32], BF16, name="ones_col")
            nc.vector.memset(ones_col[:], 1.0)
            ones_row16 = constp.tile([1, 128], F16, name="ones_row16")
            nc.vector.memset(ones_row16[:], 1.0)
            ones_row32 = constp.tile([1, 128], F32, name="ones_row32")
            nc.vector.memset(ones_row32[:], 1.0)
            eps_t = constp.tile([128, 1], F32, name="eps_t")
            nc.vector.memset(eps_t[:], EPS)

            x_cur = []
            for tt in range(2):
                xt = xresp.tile([128, H], F32, name=f"x_init{tt}", tag="x")
                nc.sync.dma_start(out=xt[:], in_=x0_e[tt * 128:(tt + 1) * 128, :])
                x_cur.append(xt)

            def layernorm_f16(xtiles, nm):
                outs = []
                for tt in range(2):
                    stats = smallp.tile([128, 2, 6], F32, name=f"st{nm}{tt}", tag="st")
                    for s in range(2):
                        nc.vector.bn_stats(out=stats[:, s, :],
                                           in_=xtiles[tt][:, s * 512:(s + 1) * 512])
                    mv = smallp.tile([128, 2], F32, name=f"mv{nm}{tt}", tag="mv")
                    nc.vector.bn_aggr(out=mv[:], in_=stats[:])
                    rstd = smallp.tile([128, 1], F32, name=f"rs{nm}{tt}", tag="rstd")
                    nc.scalar.activation(out=rstd[:], in_=mv[:, 1:2],
                                         func=mybir.ActivationFunctionType.Sqrt,
                                         bias=eps_t[:], scale=1.0)
                    nc.vector.reciprocal(out=rstd[:], in_=rstd[:])
                    h = hpoolp.tile([128, H], F16, name=f"h{nm}{tt}", tag="h")
                    nc.vector.tensor_scalar(out=h[:], in0=xtiles[tt][:],
                                            scalar1=mv[:, 0:1], scalar2=rstd[:],
                                            op0=mybir.AluOpType.subtract,
                                            op1=mybir.AluOpType.mult)
                    outs.append(h)
                return outs

            with ExitStack() as _lstk:
                _lp = lambda *a, **kw: _lstk.enter_context(tc.tile_pool(*a, **kw))
                wrowp = _lp(name="wrow", bufs=6)    # [128,2048] f16 batched weight rows
                wfp = _lp(name="wf", bufs=4)        # [128,2048] f16 wf of-pair tiles
                wprp = _lp(name="wpr", bufs=3)      # [128,2048] f16 wp row-pair tiles
                ktgp = _lp(name="ktg", bufs=4)      # [128,1024] f16 gathered k (4 cores)
                vgp = _lp(name="vg", bufs=3)        # [128,4096] bf16 gathered v (4 kbs)
                maskp = _lp(name="maskt", bufs=3)   # [128,2048] bf16 mask (8 kbs)
                hTp = _lp(name="hT", bufs=9)        # [128,256] f16 transposed acts
                qktp = _lp(name="qkt", bufs=17)     # [128,256] f16 qT/kT tiles
                vsbp = _lp(name="vsb", bufs=3)      # [128,1024] bf16 v out
                accp = _lp(name="acc", bufs=9)      # [128,512] f32 attn accums
                ctxTp = _lp(name="ctxT", bufs=9)    # [128,256] f16 ctx
                evp = _lp(name="ev", bufs=4)        # [128,256] f32 masked-score tiles
                evbp = _lp(name="evb", bufs=4)      # [128,256] bf16 exp tiles
                gtp = _lp(name="gt", bufs=18)       # [128,256] f16 mlp mid
                rbp = _lp(name="rb", bufs=2)        # [128,256] f32 recip bcast
                biasp = _lp(name="bias", bufs=2)    # [1,1024] f16 bias rows

                def transpose_h(htiles, nm):
                    hT = []
                    for hk in range(8):
                        t = hTp.tile([128, TL], F16, name=f"hT{nm}{hk}", tag="hT")
                        for tt in range(2):
                            pt = ps_sc.tile([128, 128], F16, name=f"ptr{nm}{hk}{tt}",
                                            tag="sc")
                            nc.tensor.transpose(pt[:],
                                                htiles[tt][:, hk * 128:(hk + 1) * 128],
                                                ident[:])
                            nc.vector.tensor_copy(out=t[:, tt * 128:(tt + 1) * 128],
                                                  in_=pt[:])
                        hT.append(t)
                    return hT

                def load_w2(we, l, nm):
                    # 4 batched DMAs of [128, 2048] (two 128-row blocks each),
                    # issued on the Pool engine (SWDGE) to bypass HWDGE.
                    tiles = []
                    for k2 in range(4):
                        w = wrowp.tile([128, 2 * H], F16, name=f"{nm}{l}_{k2}",
                                       tag="wrow")
                        nc.gpsimd.dma_start(
                            out=w[:],
                            in_=we[l, k2 * 256:(k2 + 1) * 256, :]
                                .rearrange("(a p) j -> p a j", p=128))
                        tiles.append(w)
                    return [tiles[k // 2][:, (k % 2) * H:(k % 2 + 1) * H]
                            for k in range(8)]

                for l in range(n_layers):
                    is_local = ATTN_LOCAL[l]

                    h1 = layernorm_f16(x_cur, f"l{l}a")
                    hT = transpose_h(h1, f"l{l}a")

                    qb_sb = smallp.tile([128, 8], F32, name=f"qb{l}", tag="qb")
                    nc.sync.dma_start(out=qb_sb[:], in_=qb_e[l])
                    kb_sb = smallp.tile([128, 8], F32, name=f"kb{l}", tag="kb")
                    nc.sync.dma_start(out=kb_sb[:], in_=kb_e[l])
                    vb_sb = biasp.tile([1, H], F16, name=f"vb{l}", tag="vb")
                    nc.sync.dma_start(out=vb_sb[:], in_=vb_e[l])
                    ob_sb = biasp.tile([1, H], F16, name=f"ob{l}", tag="ob")
                    nc.sync.dma_start(out=ob_sb[:], in_=ob_e[l])
                    fb_sb = smallp.tile([128, 32], F32, name=f"fb{l}", tag="fb")
                    nc.sync.dma_start(out=fb_sb[:], in_=fb_e[l])
                    pb_sb = biasp.tile([1, H], F16, name=f"pb{l}", tag="pb")
                    nc.sync.dma_start(out=pb_sb[:], in_=pb_e[l])

                    if not is_local:
                        # masks for all 16 key blocks in 2 batched DMAs
                        mtiles = []
                        for mb in range(2):
                            m = maskp.tile([128, 8 * TL], BF16, name=f"m{l}{mb}",
                                           tag="mask")
                            nc.sync.dma_start(
                                out=m[:],
                                in_=mg_e[mb * 8:(mb + 1) * 8]
                                    .rearrange("a p j -> p a j"))
                            mtiles.append(m)

                        def mt(kb):
                            return mtiles[kb // 8][:, (kb % 8) * TL:(kb % 8 + 1) * TL]

                    # ---- kT first so AllGather(k) overlaps v/q compute ----
                    # local layers: merged k+v bounce [2048, TL] f16
                    #   rows 0:1024   kT (f16), rows 1024:2048  v bits (bf16)
                    wkr = load_w2(wk_e, l, "wk")
                    if is_local:
                        bounce_k = dramp.tile([2 * H, TL], F16, name=f"bkv{l}",
                                              tag="bk")
                    else:
                        bounce_k = dramp.tile([H, TL], F16, name=f"bk{l}", tag="bk")
                    for of in range(8):
                        pq = ps_sc.tile([128, TL], F32, name=f"pk{l}{of}", tag="sc")
                        for k in range(8):
                            nc.tensor.matmul(pq[:], wkr[k][:, of * 128:(of + 1) * 128],
                                             hT[k][:], start=(k == 0), stop=(k == 7))
                        t = qktp.tile([128, TL], F16, name=f"kt{l}{of}", tag="qkt")
                        nc.vector.tensor_scalar_add(out=t[:], in0=pq[:],
                                                    scalar1=kb_sb[:, of:of + 1])
                        nc.sync.dma_start(out=bounce_k[of * 128:(of + 1) * 128, :],
                                          in_=t[:])
                    if not is_local:
                        gath_k = dramp.tile([NCORES * H, TL], F16, name=f"gk{l}",
                                            tag="gk", addr_space="Shared")
                        nc.gpsimd.collective_compute("AllGather",
                                                     mybir.AluOpType.bypass,
                                                     replica_groups=RG,
                                                     ins=[bounce_k[:]],
                                                     outs=[gath_k[:]])

                    # ---- v (bf16 out; ctx matmul runs bf16 at 1 cyc/row) ----
                    wvr = load_w2(wv_e, l, "wv")
                    if not is_local:
                        bounce_v = dramp.tile([TL, H], BF16, name=f"bv{l}", tag="bv")
                    for tt in range(2):
                        vt = vsbp.tile([128, H], BF16, name=f"v{l}{tt}", tag="vsb")
                        for nn in range(2):
                            pv = ps_mm.tile([128, 512], F32, name=f"pv{l}{tt}{nn}",
                                            tag="mm")
                            for k in range(8):
                                nc.tensor.matmul(pv[:], hT[k][:, tt * 128:(tt + 1) * 128],
                                                 wvr[k][:, nn * 512:(nn + 1) * 512],
                                                 start=(k == 0), stop=False)
                            nc.tensor.matmul(pv[:], ones_row16[:, 0:128],
                                             vb_sb[:, nn * 512:(nn + 1) * 512],
                                             start=False, stop=True)
                            nc.vector.tensor_copy(out=vt[:, nn * 512:(nn + 1) * 512],
                                                  in_=pv[:])
                        if is_local:
                            nc.sync.dma_start(
                                out=bounce_k[H + tt * 512:H + (tt + 1) * 512, :]
                                    .bitcast(BF16)
                                    .rearrange("(p a) j -> p a j", p=128),
                                in_=vt[:])
                        else:
                            nc.sync.dma_start(out=bounce_v[tt * 128:(tt + 1) * 128, :],
                                              in_=vt[:])
                    if is_local:
                        # two pair AllGathers: every core ends with [left, own]
                        # k/v in its designated buffer (odd cores: A, even: B)
                        gkvA = dramp.tile([2 * 2 * H, TL], F16, name=f"gkvA{l}",
                                          tag="gk", addr_space="Shared")
                        gkvB = dramp.tile([2 * 2 * H, TL], F16, name=f"gkvB{l}",
                                          tag="gv", addr_space="Shared")
                        nc.gpsimd.collective_compute(
                            "AllGather", mybir.AluOpType.bypass,
                            replica_groups=[[0, 1], [2, 3], [4, 5], [6, 7]],
                            ins=[bounce_k[:]], outs=[gkvA[:]])
                        nc.gpsimd.collective_compute(
                            "AllGather", mybir.AluOpType.bypass,
                            replica_groups=[[0, 7], [1, 2], [3, 4], [5, 6]],
                            ins=[bounce_k[:]], outs=[gkvB[:]])
                    else:
                        gath_v = dramp.tile([T, H], BF16, name=f"gv{l}", tag="gv",
                                            addr_space="Shared")
                        nc.gpsimd.collective_compute("AllGather",
                                                     mybir.AluOpType.bypass,
                                                     replica_groups=RG,
                                                     ins=[bounce_v[:]],
                                                     outs=[gath_v[:]])

                    # ---- qT (stays local) ----
                    wqr = load_w2(wq_e, l, "wq")
                    qt = []
                    for of in range(8):
                        pq = ps_sc.tile([128, TL], F32, name=f"pq{l}{of}", tag="sc")
                        for k in range(8):
                            nc.tensor.matmul(pq[:], wqr[k][:, of * 128:(of + 1) * 128],
                                             hT[k][:], start=(k == 0), stop=(k == 7))
                        t = qktp.tile([128, TL], F16, name=f"qt{l}{of}", tag="qkt")
                        nc.vector.tensor_scalar_add(out=t[:], in0=pq[:],
                                                    scalar1=qb_sb[:, of:of + 1])
                        qt.append(t)

                    # ---- attention ----
                    if is_local:
                        ctxT = [None] * HP
                        gA3 = gkvA[:].rearrange("(m p) j -> m p j", m=2)
                        gB3 = gkvB[:].rearrange("(m p) j -> m p j", m=2)
                        vloc = []
                        for X3, Xn in ((gA3, "A"), (gB3, "B")):
                            vm = []
                            for m in range(2):
                                vt2 = vgp.tile([128, 2048], BF16,
                                               name=f"vl{l}{Xn}{m}", tag="vg")
                                nc.sync.dma_start(
                                    out=vt2[:],
                                    in_=X3[m, H:2 * H, :].bitcast(BF16)
                                        .rearrange("(u p a) j -> p u a j",
                                                   u=2, p=128))
                                vm.append(vt2)
                            vloc.append(vm)
                        mloc = maskp.tile([128, 8 * TL], BF16, name=f"ml{l}",
                                          tag="mask")
                        nc.sync.dma_start(out=mloc[:],
                                          in_=ml_e[:].rearrange("a p j -> p a j"))
                        for hp in range(HP):
                            kts = []
                            for X3, Xn in ((gA3, "A"), (gB3, "B")):
                                kt = ktgp.tile([128, 512], F16,
                                               name=f"ktl{l}{hp}{Xn}", tag="ktg")
                                nc.sync.dma_start(
                                    out=kt[:],
                                    in_=X3[:, hp * 128:(hp + 1) * 128, :]
                                        .transpose([1, 0, 2]))
                                kts.append(kt)
                            pcs = ps_ctx.tile([128, 512], F32, name=f"pcl{l}{hp}",
                                              tag="ctx")
                            nc.vector.memset(pcs[:], 0.0)
                            for s in range(8):
                                X = s // 4
                                m = (s % 4) // 2
                                u = s % 2
                                ksl = slice(m * 256 + u * 128,
                                            m * 256 + (u + 1) * 128)
                                s0 = ps_sc.tile([128, TL], F32,
                                                name=f"ls0_{l}{hp}{s}", tag="sc")
                                s1 = ps_sc.tile([128, TL], F32,
                                                name=f"ls1_{l}{hp}{s}", tag="sc")
                                nc.tensor.matmul(s0[:], kts[X][0:64, ksl],
                                                 qt[hp][0:64, :],
                                                 start=True, stop=True)
                                nc.tensor.matmul(s1[:], kts[X][64:128, ksl],
                                                 qt[hp][64:128, :],
                                                 start=True, stop=True)
                                ef0 = evp.tile([128, TL], F32,
                                               name=f"lef0_{l}{hp}{s}", tag="ev")
                                ef1 = evp.tile([128, TL], F32,
                                               name=f"lef1_{l}{hp}{s}", tag="ev")
                                msl = mloc[:, s * TL:(s + 1) * TL]
                                nc.vector.tensor_tensor(out=ef0[:], in0=s0[:],
                                                        in1=msl,
                                                        op=mybir.AluOpType.add)
                                nc.vector.tensor_tensor(out=ef1[:], in0=s1[:],
                                                        in1=msl,
                                                        op=mybir.AluOpType.add)
                                e0 = evbp.tile([128, TL], BF16,
                                               name=f"le0_{l}{hp}{s}", tag="evb")
                                e1 = evbp.tile([128, TL], BF16,
                                               name=f"le1_{l}{hp}{s}", tag="evb")
                                nc.scalar.activation(out=e0[:], in_=ef0[:],
                                                     func=mybir.ActivationFunctionType.Exp)
                                nc.scalar.activation(out=e1[:], in_=ef1[:],
                                                     func=mybir.ActivationFunctionType.Exp)
                                sp = (s == 7)
                                vb0 = u * 1024 + hp * 128
                                nc.tensor.matmul(pcs[0:64, 0:TL],
                                                 vloc[X][m][:, vb0:vb0 + 64], e0[:],
                                                 start=False, stop=sp,
                                                 tile_position=(0, 0),
                                                 skip_group_check=True)
                                nc.tensor.matmul(pcs[64:128, 0:TL],
                                                 vloc[X][m][:, vb0 + 64:vb0 + 128],
                                                 e1[:],
                                                 start=False, stop=sp,
                                                 tile_position=(0, 64),
                                                 skip_group_check=True)
                                nc.tensor.matmul(pcs[0:32, TL:2 * TL], ones_col[:],
                                                 e0[:],
                                                 start=False, stop=sp,
                                                 tile_position=(0, 0),
                                                 skip_group_check=True)
                                nc.tensor.matmul(pcs[32:64, TL:2 * TL], ones_col[:],
                                                 e1[:],
                                                 start=False, stop=sp,
                                                 tile_position=(0, 32),
                                                 skip_group_check=True)
                            rsA = smallp.tile([1, TL], F32, name=f"lrsA{l}{hp}",
                                              tag="rsA")
                            rsB = smallp.tile([1, TL], F32, name=f"lrsB{l}{hp}",
                                              tag="rsB")
                            nc.vector.reciprocal(out=rsA[:],
                                                 in_=pcs[0:1, TL:2 * TL])
                            nc.vector.reciprocal(out=rsB[:],
                                                 in_=pcs[32:33, TL:2 * TL])
                            pbc = ps_sc.tile([128, TL], F32, name=f"lpbc{l}{hp}",
                                             tag="sc")
                            nc.tensor.matmul(pbc[0:64, :], ones_row32[:, 0:64],
                                             rsA[:], start=True, stop=True,
                                             tile_position=(0, 0))
                            nc.tensor.matmul(pbc[64:128, :], ones_row32[:, 0:64],
                                             rsB[:], start=True, stop=True,
                                             tile_position=(0, 64))
                            rb = rbp.tile([128, TL], F32, name=f"lrb{l}{hp}",
                                          tag="rb")
                            nc.vector.tensor_copy(out=rb[:], in_=pbc[:])
                            ct = ctxTp.tile([128, TL], F16, name=f"lct{l}{hp}",
                                            tag="ctxT")
                            nc.vector.tensor_tensor(out=ct[:], in0=pcs[:, 0:TL],
                                                    in1=rb[:],
                                                    op=mybir.AluOpType.mult)
                            ctxT[hp] = ct
                        # skip the global-attention path below
                        attn_done = True
                    else:
                        attn_done = False
                    # ---- global attention: kb-half outer, head-pair inner ----
                    if not attn_done:
                     gk3 = gath_k[:].rearrange("(c p) j -> c p j", c=NCORES)
                     acc_t = [None] * HP
                     ctxT = [None] * HP
                     for half in range(2):
                        kbs = range(half * 8, half * 8 + 8)
                        vg4 = []
                        for j in range(2):
                            q0 = half * 1024 + j * 512
                            vt4 = vgp.tile([128, 4 * H], BF16, name=f"vg{l}{half}{j}",
                                           tag="vg")
                            nc.sync.dma_start(
                                out=vt4[:],
                                in_=gath_v[q0:q0 + 512, :]
                                    .rearrange("(a p) j -> p a j", p=128))
                            vg4.append(vt4)

                        def vgs(kb, cols):
                            r = kb - half * 8
                            base = (r % 4) * H
                            return vg4[r // 4][:, base + cols.start:base + cols.stop]

                        for hp in range(HP):
                            kt4 = ktgp.tile([128, 4 * TL], F16, name=f"ktg{l}{half}{hp}",
                                            tag="ktg")
                            nc.sync.dma_start(
                                out=kt4[:],
                                in_=gk3[half * 4:(half + 1) * 4,
                                        hp * 128:(hp + 1) * 128, :]
                                    .transpose([1, 0, 2]))

                            pcs = ps_ctx.tile([128, 512], F32, name=f"pcs{l}{half}{hp}",
                                              tag="ctx")
                            # interleaved accumulation groups share this bank; a
                            # start=True would mark the whole 2KB bank row pending-
                            # zero and wipe sibling groups, so init via memset and
                            # accumulate with start=False throughout.
                            nc.vector.memset(pcs[:], 0.0)
                            for kb in kbs:
                                cc, hf = kb // 2, kb % 2
                                ksl = slice((cc - half * 4) * TL + hf * 128,
                                            (cc - half * 4) * TL + (hf + 1) * 128)
                                s0 = ps_sc.tile([128, TL], F32, name=f"s0_{l}{hp}{kb}",
                                                tag="sc")
                                s1 = ps_sc.tile([128, TL], F32, name=f"s1_{l}{hp}{kb}",
                                                tag="sc")
                                nc.tensor.matmul(s0[:], kt4[0:64, ksl], qt[hp][0:64, :],
                                                 start=True, stop=True)
                                nc.tensor.matmul(s1[:], kt4[64:128, ksl],
                                                 qt[hp][64:128, :],
                                                 start=True, stop=True)
                                ef0 = evp.tile([128, TL], F32, name=f"ef0_{l}{hp}{kb}",
                                               tag="ev")
                                ef1 = evp.tile([128, TL], F32, name=f"ef1_{l}{hp}{kb}",
                                               tag="ev")
                                nc.vector.tensor_tensor(out=ef0[:], in0=s0[:],
                                                        in1=mt(kb),
                                                        op=mybir.AluOpType.add)
                                nc.vector.tensor_tensor(out=ef1[:], in0=s1[:],
                                                        in1=mt(kb),
                                                        op=mybir.AluOpType.add)
                                e0 = evbp.tile([128, TL], BF16, name=f"e0_{l}{hp}{kb}",
                                               tag="evb")
                                e1 = evbp.tile([128, TL], BF16, name=f"e1_{l}{hp}{kb}",
                                               tag="evb")
                                nc.scalar.activation(out=e0[:], in_=ef0[:],
                                                     func=mybir.ActivationFunctionType.Exp)
                                nc.scalar.activation(out=e1[:], in_=ef1[:],
                                                     func=mybir.ActivationFunctionType.Exp)
                                sp = (kb == half * 8 + 7)
                                nc.tensor.matmul(pcs[0:64, 0:TL],
                                                 vgs(kb, slice(hp * 128, hp * 128 + 64)),
                                                 e0[:],
                                                 start=False, stop=sp,
                                                 tile_position=(0, 0),
                                                 skip_group_check=True)
                                nc.tensor.matmul(pcs[64:128, 0:TL],
                                                 vgs(kb, slice(hp * 128 + 64,
                                                               (hp + 1) * 128)),
                                                 e1[:],
                                                 start=False, stop=sp,
                                                 tile_position=(0, 64),
                                                 skip_group_check=True)
                                nc.tensor.matmul(pcs[0:32, TL:2 * TL], ones_col[:],
                                                 e0[:],
                                                 start=False, stop=sp,
                                                 tile_position=(0, 0),
                                                 skip_group_check=True)
                                nc.tensor.matmul(pcs[32:64, TL:2 * TL], ones_col[:],
                                                 e1[:],
                                                 start=False, stop=sp,
                                                 tile_position=(0, 32),
                                                 skip_group_check=True)
                            if half == 0:
                                a = accp.tile([128, 512], F32, name=f"ac{l}{hp}",
                                              tag="acc")
                                nc.vector.tensor_copy(out=a[:], in_=pcs[:])
                                acc_t[hp] = a
                            else:
                                comb = accp.tile([128, 512], F32, name=f"cb{l}{hp}",
                                                 tag="acc")
                                nc.vector.tensor_tensor(out=comb[:], in0=pcs[:],
                                                        in1=acc_t[hp][:],
                                                        op=mybir.AluOpType.add)
                                rsA = smallp.tile([1, TL], F32, name=f"rsA{l}{hp}",
                                                  tag="rsA")
                                rsB = smallp.tile([1, TL], F32, name=f"rsB{l}{hp}",
                                                  tag="rsB")
                                nc.vector.reciprocal(out=rsA[:],
                                                     in_=comb[0:1, TL:2 * TL])
                                nc.vector.reciprocal(out=rsB[:],
                                                     in_=comb[32:33, TL:2 * TL])
                                pbc = ps_sc.tile([128, TL], F32, name=f"pbc{l}{hp}",
                                                 tag="sc")
                                nc.tensor.matmul(pbc[0:64, :], ones_row32[:, 0:64],
                                                 rsA[:],
                                                 start=True, stop=True,
                                                 tile_position=(0, 0))
                                nc.tensor.matmul(pbc[64:128, :], ones_row32[:, 0:64],
                                                 rsB[:],
                                                 start=True, stop=True,
                                                 tile_position=(0, 64))
                                rb = rbp.tile([128, TL], F32, name=f"rb{l}{hp}",
                                              tag="rb")
                                nc.vector.tensor_copy(out=rb[:], in_=pbc[:])
                                ct = ctxTp.tile([128, TL], F16, name=f"ct{l}{hp}",
                                                tag="ctxT")
                                nc.vector.tensor_tensor(out=ct[:], in0=comb[:, 0:TL],
                                                        in1=rb[:],
                                                        op=mybir.AluOpType.mult)
                                ctxT[hp] = ct

                    # ---- attention out projection + residual ----
                    wor = load_w2(wo_e, l, "wo")
                    x_new = []
                    for tt in range(2):
                        xt = xresp.tile([128, H], F32, name=f"xa{l}{tt}", tag="x")
                        for nn in range(2):
                            pa = ps_mm.tile([128, 512], F32, name=f"pa{l}{tt}{nn}",
                                            tag="mm")
                            for k in range(8):
                                nc.tensor.matmul(pa[:], ctxT[k][:, tt * 128:(tt + 1) * 128],
                                                 wor[k][:, nn * 512:(nn + 1) * 512],
                                                 start=(k == 0), stop=False)
                            nc.tensor.matmul(pa[:], ones_row16[:, 0:128],
                                             ob_sb[:, nn * 512:(nn + 1) * 512],
                                             start=False, stop=True)
                            nc.vector.tensor_tensor(out=xt[:, nn * 512:(nn + 1) * 512],
                                                    in0=pa[:],
                                                    in1=x_cur[tt][:, nn * 512:(nn + 1) * 512],
                                                    op=mybir.AluOpType.add)
                        x_new.append(xt)
                    x_cur = x_new

                    # ---- MLP: fc streams of-pairs, proj accumulates across all
                    # 32 k-blocks in 4 live PSUM groups ----
                    h2 = layernorm_f16(x_cur, f"l{l}b")
                    h2T = transpose_h(h2, f"l{l}b")
                    pps = [[None, None], [None, None]]
                    for tt in range(2):
                        for nn in range(2):
                            pool = ps_ctx if tt == 0 else ps_mm
                            pps[tt][nn] = pool.tile([128, 512], F32,
                                                    name=f"pp{l}{tt}{nn}",
                                                    tag="ctx" if tt == 0 else "mm")
                    x_new = [xresp.tile([128, H], F32, name=f"xm{l}{tt}", tag="x")
                             for tt in range(2)]
                    for halfk in range(2):
                        gts = []
                        for ofp in range(8):
                            wf2 = wfp.tile([128, 2048], F16,
                                           name=f"wf{l}{halfk}{ofp}", tag="wf")
                            c0 = halfk * 2048 + ofp * 256
                            nc.gpsimd.dma_start(
                                out=wf2[:],
                                in_=wf_e[l, :, c0:c0 + 256]
                                    .rearrange("(a p) (o j) -> p a o j", p=128, o=2))
                            for o in range(2):
                                of = halfk * 16 + ofp * 2 + o
                                pf = ps_sc.tile([128, TL], F32, name=f"pf{l}{of}",
                                                tag="sc")
                                for k in range(8):
                                    nc.tensor.matmul(
                                        pf[:],
                                        wf2[:, k * 256 + o * 128:k * 256 + o * 128 + 128],
                                        h2T[k][:],
                                        start=(k == 0), stop=(k == 7))
                                g = gtp.tile([128, TL], F16, name=f"g{l}{of}", tag="g")
                                nc.scalar.activation(out=g[:], in_=pf[:],
                                                     func=mybir.ActivationFunctionType.Gelu,
                                                     bias=fb_sb[:, of:of + 1], scale=1.0)
                                gts.append(g)
                        for kk2 in range(8):
                            w2 = wprp.tile([128, 2048], F16,
                                           name=f"wp{l}{halfk}{kk2}", tag="wpr")
                            r0 = halfk * 2048 + kk2 * 256
                            nc.gpsimd.dma_start(
                                out=w2[:],
                                in_=wp_e[l, r0:r0 + 256, :]
                                    .rearrange("(a p) j -> p a j", p=128))
                            for a in range(2):
                                kk = kk2 * 2 + a
                                for tt in range(2):
                                    for nn in range(2):
                                        nc.tensor.matmul(
                                            pps[tt][nn][:],
                                            gts[kk][:, tt * 128:(tt + 1) * 128],
                                            w2[:, a * H + nn * 512:a * H + (nn + 1) * 512],
                                            start=(halfk == 0 and kk == 0),
                                            stop=False)
                    for tt in range(2):
                        for nn in range(2):
                            nc.tensor.matmul(pps[tt][nn][:], ones_row16[:, 0:128],
                                             pb_sb[:, nn * 512:(nn + 1) * 512],
                                             start=False, stop=True)
                            nc.vector.tensor_tensor(
                                out=x_new[tt][:, nn * 512:(nn + 1) * 512],
                                in0=pps[tt][nn][:],
                                in1=x_cur[tt][:, nn * 512:(nn + 1) * 512],
                                op=mybir.AluOpType.add)
                    x_cur = x_new

                if not with_logits:
                    for tt in range(2):
                        nc.sync.dma_start(out=out_e[tt * 128:(tt + 1) * 128, :],
                                          in_=x_cur[tt][:])
                    gath_x = None
                else:
                    xh = layernorm_f16(x_cur, "f")
                    xhT = transpose_h(xh, "f")
                    bounce_x = dramp.tile([H, TL], F16, name="bx", tag="bx")
                    for hk in range(8):
                        nc.sync.dma_start(out=bounce_x[hk * 128:(hk + 1) * 128, :],
                                          in_=xhT[hk][:])
                    gath_x = dramp.tile([NCORES * H, TL], F16, name="gx", tag="gx",
                                        addr_space="Shared")
                    nc.gpsimd.collective_compute("AllGather", mybir.AluOpType.bypass,
                                                 replica_groups=RG,
                                                 ins=[bounce_x[:]], outs=[gath_x[:]])

            if with_logits:
                with ExitStack() as _gstk:
                    _gp = lambda *a, **kw: _gstk.enter_context(tc.tile_pool(*a, **kw))
                    xtgp = _gp(name="xtg", bufs=8)   # [128,2048] f16 resident xT
                    lmp = _gp(name="lmp", bufs=4)    # [128,2048] f16 lm vv-pairs
                    outp = _gp(name="outp", bufs=3)  # [128,2048] f16 out rows

                    lbt_sb = smallp.tile([128, VSH // 128], F32, name="lbt", tag="lbt")
                    nc.sync.dma_start(out=lbt_sb[:], in_=lbt_e[:])
                    gx3 = gath_x[:].rearrange("(c p) j -> c p j", c=NCORES)
                    # all 8 xT tiles stay resident so lm streams exactly once;
                    # xtg[k][:, cc*256:(cc+1)*256] = gath_x core cc, k-block k
                    xtg = []
                    for k in range(8):
                        t = xtgp.tile([128, 2048], F16, name=f"xtg{k}", tag="xtg")
                        nc.sync.dma_start(
                            out=t[:],
                            in_=gx3[:, k * 128:(k + 1) * 128, :].transpose([1, 0, 2]))
                        xtg.append(t)
                    for vp in range(VSH // 256):
                        lm2 = lmp.tile([128, 2048], F16, name=f"lm{vp}", tag="lmp")
                        nc.gpsimd.dma_start(
                            out=lm2[:],
                            in_=lm_e[:, vp * 256:(vp + 1) * 256]
                                .rearrange("(a p) (o j) -> p a o j", p=128, o=2))
                        for o in range(2):
                            vv = vp * 2 + o
                            ot = outp.tile([128, 2048], F16, name=f"o{vv}", tag="outp")
                            for tc4 in range(4):
                                pl = ps_mm.tile([128, 512], F32, name=f"pl{tc4}{vv}",
                                                tag="mm")
                                for k in range(8):
                                    nc.tensor.matmul(
                                        pl[:],
                                        lm2[:, k * 256 + o * 128:k * 256 + o * 128 + 128],
                                        xtg[k][:, tc4 * 512:(tc4 + 1) * 512],
                                        start=(k == 0), stop=(k == 7))
                                nc.vector.tensor_scalar_add(
                                    out=ot[:, tc4 * 512:(tc4 + 1) * 512], in0=pl[:],
                                    scalar1=lbt_sb[:, vv:vv + 1])
                            nc.sync.dma_start(out=out_e[vv * 128:(vv + 1) * 128, :],
                                              in_=ot[:])

    nc.finalize()
    return nc


# ------------------- host-side prep & entry -------------------

def _prep_inputs(inputs, n_layers=NL, with_logits=True):
    f32 = np.float32
    f16 = np.float16
    import ml_dtypes
    bf16 = ml_dtypes.bfloat16

    ids = np.asarray(inputs["input_ids"]).reshape(-1).astype(np.int64)
    wte = np.asarray(inputs["wte"], f32)
    wpe = np.asarray(inputs["wpe"], f32)
    x0 = wte[ids] + wpe[:T]

    wq = np.empty((n_layers, H, H), f16); wk = np.empty((n_layers, H, H), f16)
    wv = np.empty((n_layers, H, H), f16); wo = np.empty((n_layers, H, H), f16)
    wf = np.empty((n_layers, H, MLP), f16); wp = np.empty((n_layers, MLP, H), f16)
    qb = np.empty((n_layers, 128, 8), f32); kbb = np.empty((n_layers, 128, 8), f32)
    vb = np.empty((n_layers, 1, H), f16); ob = np.empty((n_layers, 1, H), f16)
    fb = np.empty((n_layers, 128, 32), f32); pb = np.empty((n_layers, 1, H), f16)
    for l in range(n_layers):
        ln1w = np.asarray(inputs["ln1_w"][l], f32); ln1b = np.asarray(inputs["ln1_b"][l], f32)
        ln2w = np.asarray(inputs["ln2_w"][l], f32); ln2b = np.asarray(inputs["ln2_b"][l], f32)
        for (wdst, bdst, wname) in ((wq, qb, "q_w"), (wk, kbb, "k_w")):
            w = np.asarray(inputs[wname][l], f32)
            wdst[l] = (ln1w[:, None] * w).astype(f16)
            bdst[l] = (ln1b @ w).reshape(8, 128).T
        w = np.asarray(inputs["v_w"][l], f32)
        wv[l] = (ln1w[:, None] * w).astype(f16)
        vb[l] = (ln1b @ w)[None, :].astype(f16)
        wo[l] = np.asarray(inputs["o_w"][l], f32).astype(f16)
        ob[l] = np.asarray(inputs["o_b"][l], f32)[None, :].astype(f16)
        w = np.asarray(inputs["fc_w"][l], f32)
        wf[l] = (ln2w[:, None] * w).astype(f16)
        fbv = np.asarray(inputs["fc_b"][l], f32) + ln2b @ w
        fb[l] = fbv.reshape(32, 128).T
        wp[l] = np.asarray(inputs["proj_w"][l], f32).astype(f16)
        pb[l] = np.asarray(inputs["proj_b"][l], f32)[None, :].astype(f16)

    lnfw = np.asarray(inputs["lnf_w"], f32); lnfb = np.asarray(inputs["lnf_b"], f32)
    VP = NCORES * VSH
    lm_pad = np.zeros((VP, H), f16)
    lm_pad[:VOCAB] = (wte * lnfw[None, :]).astype(f16)
    lb_pad = np.zeros((VP,), f32)
    lb_pad[:VOCAB] = wte @ lnfb

    Bgroups = [[0, 7], [1, 2], [3, 4], [5, 6]]
    in_maps = []
    for c in range(NCORES):
        ts = c * TL
        qi = ts + np.arange(TL)[None, :]
        kj = np.arange(128)[:, None]
        mg = np.empty((KB, 128, TL), bf16)
        for kb in range(KB):
            ka = kb * 128 + kj
            causal = (ka <= qi)
            mg[kb] = np.where(causal, 0.0, -30000.0).astype(bf16)
        # local masks: 8 slots = pair-gather buffers A(0-3)/B(4-7); only the
        # designated buffer (odd cores: A, even: B) is unmasked
        mlm = np.full((8, 128, TL), -30000.0, f32)
        keepA = (c % 2 == 1)
        Agrp = [2 * (c // 2), 2 * (c // 2) + 1]
        Bgrp = next(g for g in Bgroups if c in g)
        for s in range(8):
            inA = s < 4
            if inA != keepA:
                continue
            grp = Agrp if inA else Bgrp
            owner = grp[(s % 4) // 2]
            ka = owner * 256 + (s % 2) * 128 + kj
            ok = (ka <= qi) & (qi - ka < WINDOW)
            mlm[s] = np.where(ok, 0.0, -30000.0)
        mlm = mlm.astype(bf16)
        m = {
            "x0": np.ascontiguousarray(x0[ts:ts + TL]).astype(f32),
            "wq": wq, "wk": wk, "wv": wv, "wo": wo, "wf": wf, "wp": wp,
            "qb": qb, "kb": kbb, "vb": vb, "ob": ob, "fb": fb, "pb": pb,
            "maskg": mg, "maskl": mlm,
        }
        if with_logits:
            m["lm"] = np.ascontiguousarray(lm_pad[c * VSH:(c + 1) * VSH].T)
            m["lbt"] = np.ascontiguousarray(
                lb_pad[c * VSH:(c + 1) * VSH].reshape(VSH // 128, 128).T)
        in_maps.append(m)
    return in_maps


_NC_CACHE = {}


def _get_nc(n_layers=NL, with_logits=True):
    key = (n_layers, with_logits)
    if key not in _NC_CACHE:
        _NC_CACHE[key] = build(n_layers, with_logits)
    return _NC_CACHE[key]


def run(inputs, n_layers=NL, with_logits=True, trace=False):
    nc = _get_nc(n_layers, with_logits)
    in_maps = _prep_inputs(inputs, n_layers, with_logits)
    res = run_bass_kernel_spmd(nc, in_maps, list(range(NCORES)), trace=trace)
    if with_logits:
        parts = [res.results[c]["out"] for c in range(NCORES)]   # each [VSH, T] f16
        full = np.concatenate(parts, axis=0)[:VOCAB]             # [VOCAB, T]
        out = np.ascontiguousarray(full.T.astype(np.float32))[None]  # [1, T, VOCAB]
    else:
        out = np.concatenate([res.results[c]["out"] for c in range(NCORES)], axis=0)[None]
    return out, res


def kernel(**inputs) -> np.ndarray:
    out, _ = run(inputs, NL, True, trace=False)
    return out


# revision 29
# speedup vs baseline: 1.4830x; 1.4830x over previous
"""GPT-Neo (6-layer, hidden 1024, seq 2048) forward pass on 8 TRN2 NeuronCores.

Sharding: sequence-parallel transformer (256 tokens/core) with per-layer
AllGather of K/V; attention in transposed-score orientation with max-free
softmax and additive causal/window masks fed as per-core data; vocab-sharded
tied-lm-head GEMM at the end (logits computed transposed, [vocab_shard, 2048]
per core, unsharded + f32-cast on host).

Numerics: fp16 operands for projection/MLP/logits GEMMs (fp32 PSUM), f32
residual stream, exp/attn-weights and V in bf16 for 1-cycle/row context
matmuls, softmax denominators + reciprocals in f32.

DMA: weight streams batched into [128, 2048]-shaped tiles via rearranged
access patterns and issued on the Pool engine (SWDGE) to keep the HWDGE
queue free for activation traffic; K/V/mask/logit IO batched similarly.
"""
import sys
import numpy as np

sys.path.insert(0, "/opt/trn_rl_repo")

import concourse.bass as bass
import concourse.tile as tile
from concourse import mybir, bacc
from concourse.bass_utils import run_bass_kernel_spmd
from concourse.masks import make_identity

NCORES = 8
T = 2048
TL = T // NCORES   # 256 tokens per core
H = 1024
HEADS = 16
HD = 64
MLP = 4096
NL = 6
WINDOW = 256
VOCAB = 50257
VSH = 6400         # padded per-core vocab shard (8*6400 = 51200)
EPS = 1e-5
ATTN_LOCAL = [False, True, False, True, False, True]

F16 = mybir.dt.float16
F32 = mybir.dt.float32
BF16 = mybir.dt.bfloat16

KB = T // 128      # 16 key blocks
HP = HEADS // 2    # 8 head pairs
RG = [list(range(NCORES))]


def build(n_layers=NL, with_logits=True):
    nc = bacc.Bacc(num_devices=NCORES)

    x0_e = nc.declare_dram_parameter("x0", [TL, H], F32, isOutput=False)
    wq_e = nc.declare_dram_parameter("wq", [n_layers, H, H], F16, isOutput=False)
    wk_e = nc.declare_dram_parameter("wk", [n_layers, H, H], F16, isOutput=False)
    wv_e = nc.declare_dram_parameter("wv", [n_layers, H, H], F16, isOutput=False)
    wo_e = nc.declare_dram_parameter("wo", [n_layers, H, H], F16, isOutput=False)
    wf_e = nc.declare_dram_parameter("wf", [n_layers, H, MLP], F16, isOutput=False)
    wp_e = nc.declare_dram_parameter("wp", [n_layers, MLP, H], F16, isOutput=False)
    qb_e = nc.declare_dram_parameter("qb", [n_layers, 128, 8], F32, isOutput=False)
    kb_e = nc.declare_dram_parameter("kb", [n_layers, 128, 8], F32, isOutput=False)
    vb_e = nc.declare_dram_parameter("vb", [n_layers, 1, H], F16, isOutput=False)
    ob_e = nc.declare_dram_parameter("ob", [n_layers, 1, H], F16, isOutput=False)
    fb_e = nc.declare_dram_parameter("fb", [n_layers, 128, 32], F32, isOutput=False)
    pb_e = nc.declare_dram_parameter("pb", [n_layers, 1, H], F16, isOutput=False)
    mg_e = nc.declare_dram_parameter("maskg", [KB, 128, TL], BF16, isOutput=False)
    ml_e = nc.declare_dram_parameter("maskl", [KB, 128, TL], BF16, isOutput=False)
    if with_logits:
        lm_e = nc.declare_dram_parameter("lm", [H, VSH], F16, isOutput=False)
        lbt_e = nc.declare_dram_parameter("lbt", [128, VSH // 128], F32, isOutput=False)
        out_e = nc.declare_dram_parameter("out", [VSH, T], F16, isOutput=True)
    else:
        out_e = nc.declare_dram_parameter("out", [TL, H], F32, isOutput=True)

    from contextlib import ExitStack
    with tile.TileContext(nc) as tc:
        with ExitStack() as _stk:
            _p = lambda *a, **kw: _stk.enter_context(tc.tile_pool(*a, **kw))
            constp = _p(name="const", bufs=1)
            xresp = _p(name="xres", bufs=3)     # [128,1024] f32 residual
            hpoolp = _p(name="hpool", bufs=3)   # [128,1024] f16 ln out
            smallp = _p(name="small", bufs=3)
            ps_sc = _p(name="ps_sc", bufs=3, space="PSUM")
            ps_ctx = _p(name="ps_ctx", bufs=2, space="PSUM")
            ps_mm = _p(name="ps_mm", bufs=2, space="PSUM")
            dramp = _p(name="dram", bufs=2, space="DRAM")

            ident = constp.tile([128, 128], F16, name="ident")
            make_identity(nc, ident[:])
            ones_col = constp.tile([128, 32], BF16, name="ones_col")
            nc.vector.memset(ones_col[:], 1.0)
            ones_row16 = constp.tile([1, 128], F16, name="ones_row16")
            nc.vector.memset(ones_row16[:], 1.0)
            ones_row32 = constp.tile([1, 128], F32, name="ones_row32")
            nc.vector.memset(ones_row32[:], 1.0)
            eps_t = constp.tile([128, 1], F32, name="eps_t")
            nc.vector.memset(eps_t[:], EPS)

            x_cur = []
            for tt in range(2):
                xt = xresp.tile([128, H], F32, name=f"x_init{tt}", tag="x")
                nc.sync.dma_start(out=xt[:], in_=x0_e[tt * 128:(tt + 1) * 128, :])
                x_cur.append(xt)

            def layernorm_f16(xtiles, nm):
                outs = []
                for tt in range(2):
                    stats = smallp.tile([128, 2, 6], F32, name=f"st{nm}{tt}", tag="st")
                    for s in range(2):
                        nc.vector.bn_stats(out=stats[:, s, :],
                                           in_=xtiles[tt][:, s * 512:(s + 1) * 512])
                    mv = smallp.tile([128, 2], F32, name=f"mv{nm}{tt}", tag="mv")
                    nc.vector.bn_aggr(out=mv[:], in_=stats[:])
                    rstd = smallp.tile([128, 1], F32, name=f"rs{nm}{tt}", tag="rstd")
                    nc.scalar.activation(out=rstd[:], in_=mv[:, 1:2],
                                         func=mybir.ActivationFunctionType.Sqrt,
                                         bias=eps_t[:], scale=1.0)
                    nc.vector.reciprocal(out=rstd[:], in_=rstd[:])
                    h = hpoolp.tile([128, H], F16, name=f"h{nm}{tt}", tag="h")
                    nc.vector.tensor_scalar(out=h[:], in0=xtiles[tt][:],
                                            scalar1=mv[:, 0:1], scalar2=rstd[:],
                                            op0=mybir.AluOpType.subtract,
                                            op1=mybir.AluOpType.mult)
                    outs.append(h)
                return outs

            with ExitStack() as _lstk:
                _lp = lambda *a, **kw: _lstk.enter_context(tc.tile_pool(*a, **kw))
                wrowp = _lp(name="wrow", bufs=6)    # [128,2048] f16 batched weight rows
                wfp = _lp(name="wf", bufs=4)        # [128,2048] f16 wf of-pair tiles
                wprp = _lp(name="wpr", bufs=3)      # [128,2048] f16 wp row-pair tiles
                ktgp = _lp(name="ktg", bufs=4)      # [128,1024] f16 gathered k (4 cores)
                vgp = _lp(name="vg", bufs=4)        # [128,2048] f32 gathered v (2 kbs)
                maskp = _lp(name="maskt", bufs=3)   # [128,2048] bf16 mask (8 kbs)
                hTp = _lp(name="hT", bufs=9)        # [128,256] f16 transposed acts
                qktp = _lp(name="qkt", bufs=17)     # [128,256] f16 qT/kT tiles
                vsbp = _lp(name="vsb", bufs=3)      # [128,1024] f32 v out
                accp = _lp(name="acc", bufs=9)      # [128,512] f32 attn accums
                ctxTp = _lp(name="ctxT", bufs=9)    # [128,256] f16 ctx
                evp = _lp(name="ev", bufs=3)        # [128,256] f32 masked-score tiles
                evbp = _lp(name="evb", bufs=5)      # [128,256] bf16 e hi/lo tiles
                gtp = _lp(name="gt", bufs=18)       # [128,256] f16 mlp mid
                rbp = _lp(name="rb", bufs=2)        # [128,256] f32 recip bcast
                biasp = _lp(name="bias", bufs=2)    # [1,1024] f16 bias rows

                def transpose_h(htiles, nm):
                    hT = []
                    for hk in range(8):
                        t = hTp.tile([128, TL], F16, name=f"hT{nm}{hk}", tag="hT")
                        for tt in range(2):
                            pt = ps_sc.tile([128, 128], F16, name=f"ptr{nm}{hk}{tt}",
                                            tag="sc")
                            nc.tensor.transpose(pt[:],
                                                htiles[tt][:, hk * 128:(hk + 1) * 128],
                                                ident[:])
                            nc.vector.tensor_copy(out=t[:, tt * 128:(tt + 1) * 128],
                                                  in_=pt[:])
                        hT.append(t)
                    return hT

                def load_w2(we, l, nm):
                    # 4 batched DMAs of [128, 2048] (two 128-row blocks each),
                    # issued on the Pool engine (SWDGE) to bypass HWDGE.
                    tiles = []
                    for k2 in range(4):
                        w = wrowp.tile([128, 2 * H], F16, name=f"{nm}{l}_{k2}",
                                       tag="wrow")
                        nc.gpsimd.dma_start(
                            out=w[:],
                            in_=we[l, k2 * 256:(k2 + 1) * 256, :]
                                .rearrange("(a p) j -> p a j", p=128))
                        tiles.append(w)
                    return [tiles[k // 2][:, (k % 2) * H:(k % 2 + 1) * H]
                            for k in range(8)]

                for l in range(n_layers):
                    # pairwise local-gather path disabled: Shared collective
                    # outputs need >4-core groups on this toolchain
                    is_local = False

                    h1 = layernorm_f16(x_cur, f"l{l}a")
                    hT = transpose_h(h1, f"l{l}a")

                    qb_sb = smallp.tile([128, 8], F32, name=f"qb{l}", tag="qb")
                    nc.sync.dma_start(out=qb_sb[:], in_=qb_e[l])
                    kb_sb = smallp.tile([128, 8], F32, name=f"kb{l}", tag="kb")
                    nc.sync.dma_start(out=kb_sb[:], in_=kb_e[l])
                    vb_sb = biasp.tile([1, H], F16, name=f"vb{l}", tag="vb")
                    nc.sync.dma_start(out=vb_sb[:], in_=vb_e[l])
                    ob_sb = biasp.tile([1, H], F16, name=f"ob{l}", tag="ob")
                    nc.sync.dma_start(out=ob_sb[:], in_=ob_e[l])
                    fb_sb = smallp.tile([128, 32], F32, name=f"fb{l}", tag="fb")
                    nc.sync.dma_start(out=fb_sb[:], in_=fb_e[l])
                    pb_sb = biasp.tile([1, H], F16, name=f"pb{l}", tag="pb")
                    nc.sync.dma_start(out=pb_sb[:], in_=pb_e[l])

                    if not is_local:
                        # masks for all 16 key blocks in 2 batched DMAs
                        mask_e = ml_e if ATTN_LOCAL[l] else mg_e
                        mtiles = []
                        for mb in range(2):
                            m = maskp.tile([128, 8 * TL], BF16, name=f"m{l}{mb}",
                                           tag="mask")
                            nc.sync.dma_start(
                                out=m[:],
                                in_=mask_e[mb * 8:(mb + 1) * 8]
                                    .rearrange("a p j -> p a j"))
                            mtiles.append(m)

                        def mt(kb):
                            return mtiles[kb // 8][:, (kb % 8) * TL:(kb % 8 + 1) * TL]

                    # ---- kT first so AllGather(k) overlaps v/q compute ----
                    # local layers: merged k+v bounce [2048, TL] f16
                    #   rows 0:1024   kT (f16), rows 1024:2048  v bits (bf16)
                    wkr = load_w2(wk_e, l, "wk")
                    if is_local:
                        bounce_k = dramp.tile([2 * H, TL], F16, name=f"bkv{l}",
                                              tag="bk")
                    else:
                        bounce_k = dramp.tile([H, TL], F16, name=f"bk{l}", tag="bk")
                    for of in range(8):
                        pq = ps_sc.tile([128, TL], F32, name=f"pk{l}{of}", tag="sc")
                        for k in range(8):
                            nc.tensor.matmul(pq[:], wkr[k][:, of * 128:(of + 1) * 128],
                                             hT[k][:], start=(k == 0), stop=(k == 7))
                        t = qktp.tile([128, TL], F16, name=f"kt{l}{of}", tag="qkt")
                        nc.vector.tensor_scalar_add(out=t[:], in0=pq[:],
                                                    scalar1=kb_sb[:, of:of + 1])
                        nc.sync.dma_start(out=bounce_k[of * 128:(of + 1) * 128, :],
                                          in_=t[:])
                    if not is_local:
                        gath_k = dramp.tile([NCORES * H, TL], F16, name=f"gk{l}",
                                            tag="gk", addr_space="Shared")
                        nc.gpsimd.collective_compute("AllGather",
                                                     mybir.AluOpType.bypass,
                                                     replica_groups=RG,
                                                     ins=[bounce_k[:]],
                                                     outs=[gath_k[:]])

                    # ---- v (bf16 out; ctx matmul runs bf16 at 1 cyc/row) ----
                    wvr = load_w2(wv_e, l, "wv")
                    if not is_local:
                        bounce_v = dramp.tile([TL, 2 * H], BF16, name=f"bv{l}", tag="bv")
                    for tt in range(2):
                        vt = vsbp.tile([128, 2 * H], BF16, name=f"v{l}{tt}", tag="vsb")
                        for nn in range(2):
                            pv = ps_mm.tile([128, 512], F32, name=f"pv{l}{tt}{nn}",
                                            tag="mm")
                            for k in range(8):
                                nc.tensor.matmul(pv[:], hT[k][:, tt * 128:(tt + 1) * 128],
                                                 wvr[k][:, nn * 512:(nn + 1) * 512],
                                                 start=(k == 0), stop=False)
                            nc.tensor.matmul(pv[:], ones_row16[:, 0:128],
                                             vb_sb[:, nn * 512:(nn + 1) * 512],
                                             start=False, stop=True)
                            # hi/lo bf16 split: v = hi + lo to ~17 bits
                            nc.vector.tensor_copy(out=vt[:, nn * 512:(nn + 1) * 512],
                                                  in_=pv[:])
                            nc.vector.tensor_tensor(
                                out=vt[:, H + nn * 512:H + (nn + 1) * 512],
                                in0=pv[:], in1=vt[:, nn * 512:(nn + 1) * 512],
                                op=mybir.AluOpType.subtract)
                        if is_local:
                            nc.sync.dma_start(
                                out=bounce_k[H + tt * 512:H + (tt + 1) * 512, :]
                                    .bitcast(BF16)
                                    .rearrange("(p a) j -> p a j", p=128),
                                in_=vt[:])
                        else:
                            nc.sync.dma_start(out=bounce_v[tt * 128:(tt + 1) * 128, :],
                                              in_=vt[:])
                    if is_local:
                        # two pair AllGathers: every core ends with [left, own]
                        # k/v in its designated buffer (odd cores: A, even: B)
                        gkvA = dramp.tile([2 * 2 * H, TL], F16, name=f"gkvA{l}",
                                          tag="gk", addr_space="Shared")
                        gkvB = dramp.tile([2 * 2 * H, TL], F16, name=f"gkvB{l}",
                                          tag="gv", addr_space="Shared")
                        nc.gpsimd.collective_compute(
                            "AllGather", mybir.AluOpType.bypass,
                            replica_groups=[[0, 1], [2, 3], [4, 5], [6, 7]],
                            ins=[bounce_k[:]], outs=[gkvA[:]])
                        nc.gpsimd.collective_compute(
                            "AllGather", mybir.AluOpType.bypass,
                            replica_groups=[[0, 7], [1, 2], [3, 4], [5, 6]],
                            ins=[bounce_k[:]], outs=[gkvB[:]])
                    else:
                        gath_v = dramp.tile([T, 2 * H], BF16, name=f"gv{l}", tag="gv",
                                            addr_space="Shared")
                        nc.gpsimd.collective_compute("AllGather",
                                                     mybir.AluOpType.bypass,
                                                     replica_groups=RG,
                                                     ins=[bounce_v[:]],
                                                     outs=[gath_v[:]])

                    # ---- qT (stays local) ----
                    wqr = load_w2(wq_e, l, "wq")
                    qt = []
                    for of in range(8):
                        pq = ps_sc.tile([128, TL], F32, name=f"pq{l}{of}", tag="sc")
                        for k in range(8):
                            nc.tensor.matmul(pq[:], wqr[k][:, of * 128:(of + 1) * 128],
                                             hT[k][:], start=(k == 0), stop=(k == 7))
                        t = qktp.tile([128, TL], F16, name=f"qt{l}{of}", tag="qkt")
                        nc.vector.tensor_scalar_add(out=t[:], in0=pq[:],
                                                    scalar1=qb_sb[:, of:of + 1])
                        qt.append(t)

                    # ---- attention ----
                    if is_local:
                        ctxT = [None] * HP
                        gA3 = gkvA[:].rearrange("(m p) j -> m p j", m=2)
                        gB3 = gkvB[:].rearrange("(m p) j -> m p j", m=2)
                        vloc = []
                        for X3, Xn in ((gA3, "A"), (gB3, "B")):
                            vm = []
                            for m in range(2):
                                vt2 = vgp.tile([128, 2048], BF16,
                                               name=f"vl{l}{Xn}{m}", tag="vg")
                                nc.sync.dma_start(
                                    out=vt2[:],
                                    in_=X3[m, H:2 * H, :].bitcast(BF16)
                                        .rearrange("(u p a) j -> p u a j",
                                                   u=2, p=128))
                                vm.append(vt2)
                            vloc.append(vm)
                        mloc = maskp.tile([128, 8 * TL], BF16, name=f"ml{l}",
                                          tag="mask")
                        nc.sync.dma_start(out=mloc[:],
                                          in_=ml_e[:].rearrange("a p j -> p a j"))
                        for hp in range(HP):
                            kts = []
                            for X3, Xn in ((gA3, "A"), (gB3, "B")):
                                kt = ktgp.tile([128, 512], F16,
                                               name=f"ktl{l}{hp}{Xn}", tag="ktg")
                                nc.sync.dma_start(
                                    out=kt[:],
                                    in_=X3[:, hp * 128:(hp + 1) * 128, :]
                                        .transpose([1, 0, 2]))
                                kts.append(kt)
                            pcs = ps_ctx.tile([128, 512], F32, name=f"pcl{l}{hp}",
                                              tag="ctx")
                            nc.vector.memset(pcs[:], 0.0)
                            for s in range(8):
                                X = s // 4
                                m = (s % 4) // 2
                                u = s % 2
                                ksl = slice(m * 256 + u * 128,
                                            m * 256 + (u + 1) * 128)
                                s0 = ps_sc.tile([128, TL], F32,
                                                name=f"ls0_{l}{hp}{s}", tag="sc")
                                s1 = ps_sc.tile([128, TL], F32,
                                                name=f"ls1_{l}{hp}{s}", tag="sc")
                                nc.tensor.matmul(s0[:], kts[X][0:64, ksl],
                                                 qt[hp][0:64, :],
                                                 start=True, stop=True)
                                nc.tensor.matmul(s1[:], kts[X][64:128, ksl],
                                                 qt[hp][64:128, :],
                                                 start=True, stop=True)
                                ef0 = evp.tile([128, TL], F32,
                                               name=f"lef0_{l}{hp}{s}", tag="ev")
                                ef1 = evp.tile([128, TL], F32,
                                               name=f"lef1_{l}{hp}{s}", tag="ev")
                                msl = mloc[:, s * TL:(s + 1) * TL]
                                nc.vector.tensor_tensor(out=ef0[:], in0=s0[:],
                                                        in1=msl,
                                                        op=mybir.AluOpType.add)
                                nc.vector.tensor_tensor(out=ef1[:], in0=s1[:],
                                                        in1=msl,
                                                        op=mybir.AluOpType.add)
                                e0 = evbp.tile([128, TL], BF16,
                                               name=f"le0_{l}{hp}{s}", tag="evb")
                                e1 = evbp.tile([128, TL], BF16,
                                               name=f"le1_{l}{hp}{s}", tag="evb")
                                nc.scalar.activation(out=e0[:], in_=ef0[:],
                                                     func=mybir.ActivationFunctionType.Exp)
                                nc.scalar.activation(out=e1[:], in_=ef1[:],
                                                     func=mybir.ActivationFunctionType.Exp)
                                sp = (s == 7)
                                vb0 = u * 1024 + hp * 128
                                nc.tensor.matmul(pcs[0:64, 0:TL],
                                                 vloc[X][m][:, vb0:vb0 + 64], e0[:],
                                                 start=False, stop=sp,
                                                 tile_position=(0, 0),
                                                 skip_group_check=True)
                                nc.tensor.matmul(pcs[64:128, 0:TL],
                                                 vloc[X][m][:, vb0 + 64:vb0 + 128],
                                                 e1[:],
                                                 start=False, stop=sp,
                                                 tile_position=(0, 64),
                                                 skip_group_check=True)
                                nc.tensor.matmul(pcs[0:32, TL:2 * TL], ones_col[:],
                                                 e0[:],
                                                 start=False, stop=sp,
                                                 tile_position=(0, 0),
                                                 skip_group_check=True)
                                nc.tensor.matmul(pcs[32:64, TL:2 * TL], ones_col[:],
                                                 e1[:],
                                                 start=False, stop=sp,
                                                 tile_position=(0, 32),
                                                 skip_group_check=True)
                            rsA = smallp.tile([1, TL], F32, name=f"lrsA{l}{hp}",
                                              tag="rsA")
                            rsB = smallp.tile([1, TL], F32, name=f"lrsB{l}{hp}",
                                              tag="rsB")
                            nc.vector.reciprocal(out=rsA[:],
                                                 in_=pcs[0:1, TL:2 * TL])
                            nc.vector.reciprocal(out=rsB[:],
                                                 in_=pcs[32:33, TL:2 * TL])
                            pbc = ps_sc.tile([128, TL], F32, name=f"lpbc{l}{hp}",
                                             tag="sc")
                            nc.tensor.matmul(pbc[0:64, :], ones_row32[:, 0:64],
                                             rsA[:], start=True, stop=True,
                                             tile_position=(0, 0))
                            nc.tensor.matmul(pbc[64:128, :], ones_row32[:, 0:64],
                                             rsB[:], start=True, stop=True,
                                             tile_position=(0, 64))
                            rb = rbp.tile([128, TL], F32, name=f"lrb{l}{hp}",
                                          tag="rb")
                            nc.vector.tensor_copy(out=rb[:], in_=pbc[:])
                            ct = ctxTp.tile([128, TL], F16, name=f"lct{l}{hp}",
                                            tag="ctxT")
                            nc.vector.tensor_tensor(out=ct[:], in0=pcs[:, 0:TL],
                                                    in1=rb[:],
                                                    op=mybir.AluOpType.mult)
                            ctxT[hp] = ct
                        # skip the global-attention path below
                        attn_done = True
                    else:
                        attn_done = False
                    # ---- global attention: kb-half outer, head-pair inner ----
                    if not attn_done:
                     gk3 = gath_k[:].rearrange("(c p) j -> c p j", c=NCORES)
                     acc_t = [None] * HP
                     ctxT = [None] * HP
                     for half in range(2):
                        kbs = range(half * 8, half * 8 + 8)
                        vg4 = []
                        for j in range(4):
                            q0 = half * 1024 + j * 256
                            vt4 = vgp.tile([128, 2 * 2 * H], BF16,
                                           name=f"vg{l}{half}{j}", tag="vg")
                            nc.sync.dma_start(
                                out=vt4[:],
                                in_=gath_v[q0:q0 + 256, :]
                                    .rearrange("(a p) j -> p a j", p=128))
                            vg4.append(vt4)

                        def vgs(kb, cols, lo=False):
                            r = kb - half * 8
                            base = (r % 2) * 2 * H + (H if lo else 0)
                            return vg4[r // 2][:, base + cols.start:base + cols.stop]

                        for hp in range(HP):
                            kt4 = ktgp.tile([128, 4 * TL], F16, name=f"ktg{l}{half}{hp}",
                                            tag="ktg")
                            nc.sync.dma_start(
                                out=kt4[:],
                                in_=gk3[half * 4:(half + 1) * 4,
                                        hp * 128:(hp + 1) * 128, :]
                                    .transpose([1, 0, 2]))

                            pcs = ps_ctx.tile([128, 512], F32, name=f"pcs{l}{half}{hp}",
                                              tag="ctx")
                            # interleaved accumulation groups share this bank; a
                            # start=True would mark the whole 2KB bank row pending-
                            # zero and wipe sibling groups, so init via memset and
                            # accumulate with start=False throughout.
                            nc.vector.memset(pcs[:], 0.0)
                            for kb in kbs:
                                cc, hf = kb // 2, kb % 2
                                ksl = slice((cc - half * 4) * TL + hf * 128,
                                            (cc - half * 4) * TL + (hf + 1) * 128)
                                s0 = ps_sc.tile([128, TL], F32, name=f"s0_{l}{hp}{kb}",
                                                tag="sc")
                                s1 = ps_sc.tile([128, TL], F32, name=f"s1_{l}{hp}{kb}",
                                                tag="sc")
                                nc.tensor.matmul(s0[:], kt4[0:64, ksl], qt[hp][0:64, :],
                                                 start=True, stop=True)
                                nc.tensor.matmul(s1[:], kt4[64:128, ksl],
                                                 qt[hp][64:128, :],
                                                 start=True, stop=True)
                                ef0 = evp.tile([128, TL], F32, name=f"ef0_{l}{hp}{kb}",
                                               tag="ev")
                                ef1 = evp.tile([128, TL], F32, name=f"ef1_{l}{hp}{kb}",
                                               tag="ev")
                                nc.vector.tensor_tensor(out=ef0[:], in0=s0[:],
                                                        in1=mt(kb),
                                                        op=mybir.AluOpType.add)
                                nc.vector.tensor_tensor(out=ef1[:], in0=s1[:],
                                                        in1=mt(kb),
                                                        op=mybir.AluOpType.add)
                                nc.scalar.activation(out=ef0[:], in_=ef0[:],
                                                     func=mybir.ActivationFunctionType.Exp)
                                nc.scalar.activation(out=ef1[:], in_=ef1[:],
                                                     func=mybir.ActivationFunctionType.Exp)
                                eh0 = evbp.tile([128, TL], BF16,
                                                name=f"eh0_{l}{hp}{kb}", tag="evb")
                                el0 = evbp.tile([128, TL], BF16,
                                                name=f"el0_{l}{hp}{kb}", tag="evb")
                                eh1 = evbp.tile([128, TL], BF16,
                                                name=f"eh1_{l}{hp}{kb}", tag="evb")
                                el1 = evbp.tile([128, TL], BF16,
                                                name=f"el1_{l}{hp}{kb}", tag="evb")
                                nc.vector.tensor_copy(out=eh0[:], in_=ef0[:])
                                nc.vector.tensor_tensor(out=el0[:], in0=ef0[:],
                                                        in1=eh0[:],
                                                        op=mybir.AluOpType.subtract)
                                nc.vector.tensor_copy(out=eh1[:], in_=ef1[:])
                                nc.vector.tensor_tensor(out=el1[:], in0=ef1[:],
                                                        in1=eh1[:],
                                                        op=mybir.AluOpType.subtract)
                                sp = (kb == half * 8 + 7)
                                slA = slice(hp * 128, hp * 128 + 64)
                                slB = slice(hp * 128 + 64, (hp + 1) * 128)
                                for (vlo, ee) in ((False, eh0), (False, el0),
                                                  (True, eh0)):
                                    nc.tensor.matmul(pcs[0:64, 0:TL],
                                                     vgs(kb, slA, lo=vlo), ee[:],
                                                     start=False,
                                                     stop=(sp and vlo),
                                                     tile_position=(0, 0),
                                                     skip_group_check=True)
                                for (vlo, ee) in ((False, eh1), (False, el1),
                                                  (True, eh1)):
                                    nc.tensor.matmul(pcs[64:128, 0:TL],
                                                     vgs(kb, slB, lo=vlo), ee[:],
                                                     start=False,
                                                     stop=(sp and vlo),
                                                     tile_position=(0, 64),
                                                     skip_group_check=True)
                                nc.tensor.matmul(pcs[0:32, TL:2 * TL], ones_col[:],
                                                 eh0[:], start=False, stop=False,
                                                 tile_position=(0, 0),
                                                 skip_group_check=True)
                                nc.tensor.matmul(pcs[0:32, TL:2 * TL], ones_col[:],
                                                 el0[:], start=False, stop=sp,
                                                 tile_position=(0, 0),
                                                 skip_group_check=True)
                                nc.tensor.matmul(pcs[32:64, TL:2 * TL], ones_col[:],
                                                 eh1[:], start=False, stop=False,
                                                 tile_position=(0, 32),
                                                 skip_group_check=True)
                                nc.tensor.matmul(pcs[32:64, TL:2 * TL], ones_col[:],
                                                 el1[:], start=False, stop=sp,
                                                 tile_position=(0, 32),
                                                 skip_group_check=True)
                            if half == 0:
                                a = accp.tile([128, 512], F32, name=f"ac{l}{hp}",
                                              tag="acc")
                                nc.vector.tensor_copy(out=a[:], in_=pcs[:])
                                acc_t[hp] = a
                            else:
                                comb = accp.tile([128, 512], F32, name=f"cb{l}{hp}",
                                                 tag="acc")
                                nc.vector.tensor_tensor(out=comb[:], in0=pcs[:],
                                                        in1=acc_t[hp][:],
                                                        op=mybir.AluOpType.add)
                                rsA = smallp.tile([1, TL], F32, name=f"rsA{l}{hp}",
                                                  tag="rsA")
                                rsB = smallp.tile([1, TL], F32, name=f"rsB{l}{hp}",
                                                  tag="rsB")
                                nc.vector.reciprocal(out=rsA[:],
                                                     in_=comb[0:1, TL:2 * TL])
                                nc.vector.reciprocal(out=rsB[:],
                                                     in_=comb[32:33, TL:2 * TL])
                                pbc = ps_sc.tile([128, TL], F32, name=f"pbc{l}{hp}",
                                                 tag="sc")
                                nc.tensor.matmul(pbc[0:64, :], ones_row32[:, 0:64],
                                                 rsA[:],
                                                 start=True, stop=True,
                                                 tile_position=(0, 0))
                                nc.tensor.matmul(pbc[64:128, :], ones_row32[:, 0:64],
                                                 rsB[:],
                                                 start=True, stop=True,
                                                 tile_position=(0, 64))
                                rb = rbp.tile([128, TL], F32, name=f"rb{l}{hp}",
                                              tag="rb")
                                nc.vector.tensor_copy(out=rb[:], in_=pbc[:])
                                ct = ctxTp.tile([128, TL], F16, name=f"ct{l}{hp}",
                                                tag="ctxT")
                                nc.vector.tensor_tensor(out=ct[:], in0=comb[:, 0:TL],
                                                        in1=rb[:],
                                                        op=mybir.AluOpType.mult)
                                ctxT[hp] = ct

                    # ---- attention out projection + residual ----
                    wor = load_w2(wo_e, l, "wo")
                    x_new = []
                    for tt in range(2):
                        xt = xresp.tile([128, H], F32, name=f"xa{l}{tt}", tag="x")
                        for nn in range(2):
                            pa = ps_mm.tile([128, 512], F32, name=f"pa{l}{tt}{nn}",
                                            tag="mm")
                            for k in range(8):
                                nc.tensor.matmul(pa[:], ctxT[k][:, tt * 128:(tt + 1) * 128],
                                                 wor[k][:, nn * 512:(nn + 1) * 512],
                                                 start=(k == 0), stop=False)
                            nc.tensor.matmul(pa[:], ones_row16[:, 0:128],
                                             ob_sb[:, nn * 512:(nn + 1) * 512],
                                             start=False, stop=True)
                            nc.vector.tensor_tensor(out=xt[:, nn * 512:(nn + 1) * 512],
                                                    in0=pa[:],
                                                    in1=x_cur[tt][:, nn * 512:(nn + 1) * 512],
                                                    op=mybir.AluOpType.add)
                        x_new.append(xt)
                    x_cur = x_new

                    # ---- MLP: fc streams of-pairs, proj accumulates across all
                    # 32 k-blocks in 4 live PSUM groups ----
                    h2 = layernorm_f16(x_cur, f"l{l}b")
                    h2T = transpose_h(h2, f"l{l}b")
                    pps = [[None, None], [None, None]]
                    for tt in range(2):
                        for nn in range(2):
                            pool = ps_ctx if tt == 0 else ps_mm
                            pps[tt][nn] = pool.tile([128, 512], F32,
                                                    name=f"pp{l}{tt}{nn}",
                                                    tag="ctx" if tt == 0 else "mm")
                    x_new = [xresp.tile([128, H], F32, name=f"xm{l}{tt}", tag="x")
                             for tt in range(2)]
                    for halfk in range(2):
                        gts = []
                        for ofp in range(8):
                            wf2 = wfp.tile([128, 2048], F16,
                                           name=f"wf{l}{halfk}{ofp}", tag="wf")
                            c0 = halfk * 2048 + ofp * 256
                            nc.gpsimd.dma_start(
                                out=wf2[:],
                                in_=wf_e[l, :, c0:c0 + 256]
                                    .rearrange("(a p) (o j) -> p a o j", p=128, o=2))
                            for o in range(2):
                                of = halfk * 16 + ofp * 2 + o
                                pf = ps_sc.tile([128, TL], F32, name=f"pf{l}{of}",
                                                tag="sc")
                                for k in range(8):
                                    nc.tensor.matmul(
                                        pf[:],
                                        wf2[:, k * 256 + o * 128:k * 256 + o * 128 + 128],
                                        h2T[k][:],
                                        start=(k == 0), stop=(k == 7))
                                g = gtp.tile([128, TL], F16, name=f"g{l}{of}", tag="g")
                                nc.scalar.activation(out=g[:], in_=pf[:],
                                                     func=mybir.ActivationFunctionType.Gelu,
                                                     bias=fb_sb[:, of:of + 1], scale=1.0)
                                gts.append(g)
                        for kk2 in range(8):
                            w2 = wprp.tile([128, 2048], F16,
                                           name=f"wp{l}{halfk}{kk2}", tag="wpr")
                            r0 = halfk * 2048 + kk2 * 256
                            nc.gpsimd.dma_start(
                                out=w2[:],
                                in_=wp_e[l, r0:r0 + 256, :]
                                    .rearrange("(a p) j -> p a j", p=128))
                            for a in range(2):
                                kk = kk2 * 2 + a
                                for tt in range(2):
                                    for nn in range(2):
                                        nc.tensor.matmul(
                                            pps[tt][nn][:],
                                            gts[kk][:, tt * 128:(tt + 1) * 128],
                                            w2[:, a * H + nn * 512:a * H + (nn + 1) * 512],
                                            start=(halfk == 0 and kk == 0),
                                            stop=False)
                    for tt in range(2):
                        for nn in range(2):
                            nc.tensor.matmul(pps[tt][nn][:], ones_row16[:, 0:128],
                                             pb_sb[:, nn * 512:(nn + 1) * 512],
                                             start=False, stop=True)
                            nc.vector.tensor_tensor(
                                out=x_new[tt][:, nn * 512:(nn + 1) * 512],
                                in0=pps[tt][nn][:],
                                in1=x_cur[tt][:, nn * 512:(nn + 1) * 512],
                                op=mybir.AluOpType.add)
                    x_cur = x_new

                if not with_logits:
                    for tt in range(2):
                        nc.sync.dma_start(out=out_e[tt * 128:(tt + 1) * 128, :],
                                          in_=x_cur[tt][:])
                    gath_x = None
                else:
                    xh = layernorm_f16(x_cur, "f")
                    xhT = transpose_h(xh, "f")
                    bounce_x = dramp.tile([H, TL], F16, name="bx", tag="bx")
                    for hk in range(8):
                        nc.sync.dma_start(out=bounce_x[hk * 128:(hk + 1) * 128, :],
                                          in_=xhT[hk][:])
                    gath_x = dramp.tile([NCORES * H, TL], F16, name="gx", tag="gx",
                                        addr_space="Shared")
                    nc.gpsimd.collective_compute("AllGather", mybir.AluOpType.bypass,
                                                 replica_groups=RG,
                                                 ins=[bounce_x[:]], outs=[gath_x[:]])

            if with_logits:
                with ExitStack() as _gstk:
                    _gp = lambda *a, **kw: _gstk.enter_context(tc.tile_pool(*a, **kw))
                    xtgp = _gp(name="xtg", bufs=8)   # [128,2048] f16 resident xT
                    lmp = _gp(name="lmp", bufs=4)    # [128,2048] f16 lm vv-pairs
                    outp = _gp(name="outp", bufs=3)  # [128,2048] f16 out rows

                    lbt_sb = smallp.tile([128, VSH // 128], F32, name="lbt", tag="lbt")
                    nc.sync.dma_start(out=lbt_sb[:], in_=lbt_e[:])
                    gx3 = gath_x[:].rearrange("(c p) j -> c p j", c=NCORES)
                    # all 8 xT tiles stay resident so lm streams exactly once;
                    # xtg[k][:, cc*256:(cc+1)*256] = gath_x core cc, k-block k
                    xtg = []
                    for k in range(8):
                        t = xtgp.tile([128, 2048], F16, name=f"xtg{k}", tag="xtg")
                        nc.sync.dma_start(
                            out=t[:],
                            in_=gx3[:, k * 128:(k + 1) * 128, :].transpose([1, 0, 2]))
                        xtg.append(t)
                    for vp in range(VSH // 256):
                        lm2 = lmp.tile([128, 2048], F16, name=f"lm{vp}", tag="lmp")
                        nc.gpsimd.dma_start(
                            out=lm2[:],
                            in_=lm_e[:, vp * 256:(vp + 1) * 256]
                                .rearrange("(a p) (o j) -> p a o j", p=128, o=2))
                        for o in range(2):
                            vv = vp * 2 + o
                            ot = outp.tile([128, 2048], F16, name=f"o{vv}", tag="outp")
                            for tc4 in range(4):
                                pl = ps_mm.tile([128, 512], F32, name=f"pl{tc4}{vv}",
                                                tag="mm")
                                for k in range(8):
                                    nc.tensor.matmul(
                                        pl[:],
                                        lm2[:, k * 256 + o * 128:k * 256 + o * 128 + 128],
                                        xtg[k][:, tc4 * 512:(tc4 + 1) * 512],
                                        start=(k == 0), stop=(k == 7))
                                nc.vector.tensor_scalar_add(
                                    out=ot[:, tc4 * 512:(tc4 + 1) * 512], in0=pl[:],
                                    scalar1=lbt_sb[:, vv:vv + 1])
                            nc.sync.dma_start(out=out_e[vv * 128:(vv + 1) * 128, :],
                                              in_=ot[:])

    nc.finalize()
    return nc


# ------------------- host-side prep & entry -------------------

def _prep_inputs(inputs, n_layers=NL, with_logits=True):
    f32 = np.float32
    f16 = np.float16
    import ml_dtypes
    bf16 = ml_dtypes.bfloat16

    ids = np.asarray(inputs["input_ids"]).reshape(-1).astype(np.int64)
    wte = np.asarray(inputs["wte"], f32)
    wpe = np.asarray(inputs["wpe"], f32)
    x0 = wte[ids] + wpe[:T]

    wq = np.empty((n_layers, H, H), f16); wk = np.empty((n_layers, H, H), f16)
    wv = np.empty((n_layers, H, H), f16); wo = np.empty((n_layers, H, H), f16)
    wf = np.empty((n_layers, H, MLP), f16); wp = np.empty((n_layers, MLP, H), f16)
    qb = np.empty((n_layers, 128, 8), f32); kbb = np.empty((n_layers, 128, 8), f32)
    vb = np.empty((n_layers, 1, H), f16); ob = np.empty((n_layers, 1, H), f16)
    fb = np.empty((n_layers, 128, 32), f32); pb = np.empty((n_layers, 1, H), f16)
    for l in range(n_layers):
        ln1w = np.asarray(inputs["ln1_w"][l], f32); ln1b = np.asarray(inputs["ln1_b"][l], f32)
        ln2w = np.asarray(inputs["ln2_w"][l], f32); ln2b = np.asarray(inputs["ln2_b"][l], f32)
        for (wdst, bdst, wname) in ((wq, qb, "q_w"), (wk, kbb, "k_w")):
            w = np.asarray(inputs[wname][l], f32)
            wdst[l] = (ln1w[:, None] * w).astype(f16)
            bdst[l] = (ln1b @ w).reshape(8, 128).T
        w = np.asarray(inputs["v_w"][l], f32)
        wv[l] = (ln1w[:, None] * w).astype(f16)
        vb[l] = (ln1b @ w)[None, :].astype(f16)
        wo[l] = np.asarray(inputs["o_w"][l], f32).astype(f16)
        ob[l] = np.asarray(inputs["o_b"][l], f32)[None, :].astype(f16)
        w = np.asarray(inputs["fc_w"][l], f32)
        wf[l] = (ln2w[:, None] * w).astype(f16)
        fbv = np.asarray(inputs["fc_b"][l], f32) + ln2b @ w
        fb[l] = fbv.reshape(32, 128).T
        wp[l] = np.asarray(inputs["proj_w"][l], f32).astype(f16)
        pb[l] = np.asarray(inputs["proj_b"][l], f32)[None, :].astype(f16)

    lnfw = np.asarray(inputs["lnf_w"], f32); lnfb = np.asarray(inputs["lnf_b"], f32)
    VP = NCORES * VSH
    lm_pad = np.zeros((VP, H), f16)
    lm_pad[:VOCAB] = (wte * lnfw[None, :]).astype(f16)
    lb_pad = np.zeros((VP,), f32)
    lb_pad[:VOCAB] = wte @ lnfb

    in_maps = []
    for c in range(NCORES):
        ts = c * TL
        qi = ts + np.arange(TL)[None, :]
        kj = np.arange(128)[:, None]
        mg = np.empty((KB, 128, TL), bf16)
        mlm = np.empty((KB, 128, TL), bf16)
        for kb in range(KB):
            ka = kb * 128 + kj
            causal = (ka <= qi)
            mg[kb] = np.where(causal, 0.0, -30000.0).astype(bf16)
            mlm[kb] = np.where(causal & (qi - ka < WINDOW), 0.0, -30000.0).astype(bf16)
        m = {
            "x0": np.ascontiguousarray(x0[ts:ts + TL]).astype(f32),
            "wq": wq, "wk": wk, "wv": wv, "wo": wo, "wf": wf, "wp": wp,
            "qb": qb, "kb": kbb, "vb": vb, "ob": ob, "fb": fb, "pb": pb,
            "maskg": mg, "maskl": mlm,
        }
        if with_logits:
            m["lm"] = np.ascontiguousarray(lm_pad[c * VSH:(c + 1) * VSH].T)
            m["lbt"] = np.ascontiguousarray(
                lb_pad[c * VSH:(c + 1) * VSH].reshape(VSH // 128, 128).T)
        in_maps.append(m)
    return in_maps


_NC_CACHE = {}


def _get_nc(n_layers=NL, with_logits=True):
    key = (n_layers, with_logits)
    if key not in _NC_CACHE:
        _NC_CACHE[key] = build(n_layers, with_logits)
    return _NC_CACHE[key]


def run(inputs, n_layers=NL, with_logits=True, trace=False):
    nc = _get_nc(n_layers, with_logits)
    in_maps = _prep_inputs(inputs, n_layers, with_logits)
    res = run_bass_kernel_spmd(nc, in_maps, list(range(NCORES)), trace=trace)
    if with_logits:
        parts = [res.results[c]["out"] for c in range(NCORES)]   # each [VSH, T] f16
        full = np.concatenate(parts, axis=0)[:VOCAB]             # [VOCAB, T]
        out = np.ascontiguousarray(full.T.astype(np.float32))[None]  # [1, T, VOCAB]
    else:
        out = np.concatenate([res.results[c]["out"] for c in range(NCORES)], axis=0)[None]
    return out, res


def kernel(**inputs) -> np.ndarray:
    out, _ = run(inputs, NL, True, trace=False)
    return out


# revision 30
# speedup vs baseline: 2.2409x; 1.5110x over previous
"""GPT-Neo (6-layer, hidden 1024, seq 2048) forward pass on 8 TRN2 NeuronCores.

Sharding: sequence-parallel transformer (256 tokens/core) with per-layer
AllGather of K/V; attention in transposed-score orientation with max-free
softmax and additive causal/window masks fed as per-core data; vocab-sharded
tied-lm-head GEMM at the end (logits computed transposed, [vocab_shard, 2048]
per core, unsharded + f32-cast on host).

Numerics: fp16 operands for projection/MLP/logits GEMMs (fp32 PSUM), f32
residual stream. Attention context matmuls use an error-free bf16 hi/lo
split of both exp-weights and V (vh*eh + vh*el + vl*eh, ~17-bit effective
mantissa at 1 cycle/row instead of fp32's 4); softmax denominators and
reciprocals stay f32.

DMA: weight streams batched into [128, 2048]-shaped tiles via rearranged
access patterns and issued on the Pool engine (SWDGE) to keep the HWDGE
queue free for activation traffic; K/V/mask/logit IO batched similarly.
"""
import sys
import numpy as np

sys.path.insert(0, "/opt/trn_rl_repo")

import concourse.bass as bass
import concourse.tile as tile
from concourse import mybir, bacc
from concourse.bass_utils import run_bass_kernel_spmd
from concourse.masks import make_identity

NCORES = 8
T = 2048
TL = T // NCORES   # 256 tokens per core
H = 1024
HEADS = 16
HD = 64
MLP = 4096
NL = 6
WINDOW = 256
VOCAB = 50257
VSH = 6400         # padded per-core vocab shard (8*6400 = 51200)
EPS = 1e-5
ATTN_LOCAL = [False, True, False, True, False, True]

F16 = mybir.dt.float16
F32 = mybir.dt.float32
BF16 = mybir.dt.bfloat16

KB = T // 128      # 16 key blocks
HP = HEADS // 2    # 8 head pairs
RG = [list(range(NCORES))]


def build(n_layers=NL, with_logits=True):
    nc = bacc.Bacc(num_devices=NCORES)

    x0_e = nc.declare_dram_parameter("x0", [TL, H], F32, isOutput=False)
    wq_e = nc.declare_dram_parameter("wq", [n_layers, H, H], F16, isOutput=False)
    wk_e = nc.declare_dram_parameter("wk", [n_layers, H, H], F16, isOutput=False)
    wv_e = nc.declare_dram_parameter("wv", [n_layers, H, H], F16, isOutput=False)
    wo_e = nc.declare_dram_parameter("wo", [n_layers, H, H], F16, isOutput=False)
    wf_e = nc.declare_dram_parameter("wf", [n_layers, H, MLP], F16, isOutput=False)
    wp_e = nc.declare_dram_parameter("wp", [n_layers, MLP, H], F16, isOutput=False)
    qb_e = nc.declare_dram_parameter("qb", [n_layers, 128, 8], F32, isOutput=False)
    kb_e = nc.declare_dram_parameter("kb", [n_layers, 128, 8], F32, isOutput=False)
    vb_e = nc.declare_dram_parameter("vb", [n_layers, 1, H], F16, isOutput=False)
    ob_e = nc.declare_dram_parameter("ob", [n_layers, 1, H], F16, isOutput=False)
    fb_e = nc.declare_dram_parameter("fb", [n_layers, 128, 32], F32, isOutput=False)
    pb_e = nc.declare_dram_parameter("pb", [n_layers, 1, H], F16, isOutput=False)
    mg_e = nc.declare_dram_parameter("maskg", [KB, 128, TL], BF16, isOutput=False)
    ml_e = nc.declare_dram_parameter("maskl", [KB, 128, TL], BF16, isOutput=False)
    if with_logits:
        lm_e = nc.declare_dram_parameter("lm", [H, VSH], F16, isOutput=False)
        lbt_e = nc.declare_dram_parameter("lbt", [128, VSH // 128], F32, isOutput=False)
        out_e = nc.declare_dram_parameter("out", [VSH, T], F16, isOutput=True)
    else:
        out_e = nc.declare_dram_parameter("out", [TL, H], F32, isOutput=True)

    from contextlib import ExitStack
    with tile.TileContext(nc) as tc:
        with ExitStack() as _stk:
            _p = lambda *a, **kw: _stk.enter_context(tc.tile_pool(*a, **kw))
            constp = _p(name="const", bufs=1)
            xresp = _p(name="xres", bufs=3)     # [128,1024] f32 residual
            hpoolp = _p(name="hpool", bufs=3)   # [128,1024] f16 ln out
            smallp = _p(name="small", bufs=3)
            ps_sc = _p(name="ps_sc", bufs=3, space="PSUM")
            ps_ctx = _p(name="ps_ctx", bufs=2, space="PSUM")
            ps_mm = _p(name="ps_mm", bufs=2, space="PSUM")
            dramp = _p(name="dram", bufs=2, space="DRAM")

            ident = constp.tile([128, 128], F16, name="ident")
            make_identity(nc, ident[:])
            ones_col = constp.tile([128, 32], BF16, name="ones_col")
            nc.vector.memset(ones_col[:], 1.0)
            ones_row16 = constp.tile([1, 128], F16, name="ones_row16")
            nc.vector.memset(ones_row16[:], 1.0)
            ones_row32 = constp.tile([1, 128], F32, name="ones_row32")
            nc.vector.memset(ones_row32[:], 1.0)
            eps_t = constp.tile([128, 1], F32, name="eps_t")
            nc.vector.memset(eps_t[:], EPS)

            x_cur = []
            for tt in range(2):
                xt = xresp.tile([128, H], F32, name=f"x_init{tt}", tag="x")
                nc.sync.dma_start(out=xt[:], in_=x0_e[tt * 128:(tt + 1) * 128, :])
                x_cur.append(xt)

            def layernorm_f16(xtiles, nm):
                outs = []
                for tt in range(2):
                    stats = smallp.tile([128, 2, 6], F32, name=f"st{nm}{tt}", tag="st")
                    for s in range(2):
                        nc.vector.bn_stats(out=stats[:, s, :],
                                           in_=xtiles[tt][:, s * 512:(s + 1) * 512])
                    mv = smallp.tile([128, 2], F32, name=f"mv{nm}{tt}", tag="mv")
                    nc.vector.bn_aggr(out=mv[:], in_=stats[:])
                    rstd = smallp.tile([128, 1], F32, name=f"rs{nm}{tt}", tag="rstd")
                    nc.scalar.activation(out=rstd[:], in_=mv[:, 1:2],
                                         func=mybir.ActivationFunctionType.Sqrt,
                                         bias=eps_t[:], scale=1.0)
                    nc.vector.reciprocal(out=rstd[:], in_=rstd[:])
                    h = hpoolp.tile([128, H], F16, name=f"h{nm}{tt}", tag="h")
                    nc.vector.tensor_scalar(out=h[:], in0=xtiles[tt][:],
                                            scalar1=mv[:, 0:1], scalar2=rstd[:],
                                            op0=mybir.AluOpType.subtract,
                                            op1=mybir.AluOpType.mult)
                    outs.append(h)
                return outs

            with ExitStack() as _lstk:
                _lp = lambda *a, **kw: _lstk.enter_context(tc.tile_pool(*a, **kw))
                wrowp = _lp(name="wrow", bufs=6)    # [128,2048] f16 batched weight rows
                wfp = _lp(name="wf", bufs=4)        # [128,2048] f16 wf of-pair tiles
                wprp = _lp(name="wpr", bufs=3)      # [128,2048] f16 wp row-pair tiles
                ktgp = _lp(name="ktg", bufs=4)      # [128,1024] f16 gathered k (4 cores)
                vgp = _lp(name="vg", bufs=4)        # [128,2048] f32 gathered v (2 kbs)
                maskp = _lp(name="maskt", bufs=3)   # [128,2048] bf16 mask (8 kbs)
                hTp = _lp(name="hT", bufs=9)        # [128,256] f16 transposed acts
                qktp = _lp(name="qkt", bufs=17)     # [128,256] f16 qT/kT tiles
                vsbp = _lp(name="vsb", bufs=3)      # [128,1024] f32 v out
                accp = _lp(name="acc", bufs=9)      # [128,512] f32 attn accums
                ctxTp = _lp(name="ctxT", bufs=9)    # [128,256] f16 ctx
                evp = _lp(name="ev", bufs=3)        # [128,256] f32 masked-score tiles
                evbp = _lp(name="evb", bufs=5)      # [128,256] bf16 e hi/lo tiles
                gtp = _lp(name="gt", bufs=18)       # [128,256] f16 mlp mid
                rbp = _lp(name="rb", bufs=2)        # [128,256] f32 recip bcast
                biasp = _lp(name="bias", bufs=2)    # [1,1024] f16 bias rows

                def transpose_h(htiles, nm):
                    hT = []
                    for hk in range(8):
                        t = hTp.tile([128, TL], F16, name=f"hT{nm}{hk}", tag="hT")
                        for tt in range(2):
                            pt = ps_sc.tile([128, 128], F16, name=f"ptr{nm}{hk}{tt}",
                                            tag="sc")
                            nc.tensor.transpose(pt[:],
                                                htiles[tt][:, hk * 128:(hk + 1) * 128],
                                                ident[:])
                            nc.vector.tensor_copy(out=t[:, tt * 128:(tt + 1) * 128],
                                                  in_=pt[:])
                        hT.append(t)
                    return hT

                def load_w2(we, l, nm):
                    # 4 batched DMAs of [128, 2048] (two 128-row blocks each),
                    # issued on the Pool engine (SWDGE) to bypass HWDGE.
                    tiles = []
                    for k2 in range(4):
                        w = wrowp.tile([128, 2 * H], F16, name=f"{nm}{l}_{k2}",
                                       tag="wrow")
                        nc.gpsimd.dma_start(
                            out=w[:],
                            in_=we[l, k2 * 256:(k2 + 1) * 256, :]
                                .rearrange("(a p) j -> p a j", p=128))
                        tiles.append(w)
                    return [tiles[k // 2][:, (k % 2) * H:(k % 2 + 1) * H]
                            for k in range(8)]

                for l in range(n_layers):
                    # pairwise local-gather path disabled: Shared collective
                    # outputs need >4-core groups on this toolchain
                    is_local = False

                    h1 = layernorm_f16(x_cur, f"l{l}a")
                    hT = transpose_h(h1, f"l{l}a")

                    qb_sb = smallp.tile([128, 8], F32, name=f"qb{l}", tag="qb")
                    nc.sync.dma_start(out=qb_sb[:], in_=qb_e[l])
                    kb_sb = smallp.tile([128, 8], F32, name=f"kb{l}", tag="kb")
                    nc.sync.dma_start(out=kb_sb[:], in_=kb_e[l])
                    vb_sb = biasp.tile([1, H], F16, name=f"vb{l}", tag="vb")
                    nc.sync.dma_start(out=vb_sb[:], in_=vb_e[l])
                    ob_sb = biasp.tile([1, H], F16, name=f"ob{l}", tag="ob")
                    nc.sync.dma_start(out=ob_sb[:], in_=ob_e[l])
                    fb_sb = smallp.tile([128, 32], F32, name=f"fb{l}", tag="fb")
                    nc.sync.dma_start(out=fb_sb[:], in_=fb_e[l])
                    pb_sb = biasp.tile([1, H], F16, name=f"pb{l}", tag="pb")
                    nc.sync.dma_start(out=pb_sb[:], in_=pb_e[l])

                    if not is_local:
                        # masks for all 16 key blocks in 2 batched DMAs
                        mask_e = ml_e if ATTN_LOCAL[l] else mg_e
                        mtiles = []
                        for mb in range(2):
                            m = maskp.tile([128, 8 * TL], BF16, name=f"m{l}{mb}",
                                           tag="mask")
                            nc.sync.dma_start(
                                out=m[:],
                                in_=mask_e[mb * 8:(mb + 1) * 8]
                                    .rearrange("a p j -> p a j"))
                            mtiles.append(m)

                        def mt(kb):
                            return mtiles[kb // 8][:, (kb % 8) * TL:(kb % 8 + 1) * TL]

                    # ---- kT first so AllGather(k) overlaps v/q compute ----
                    # local layers: merged k+v bounce [2048, TL] f16
                    #   rows 0:1024   kT (f16), rows 1024:2048  v bits (bf16)
                    wkr = load_w2(wk_e, l, "wk")
                    if is_local:
                        bounce_k = dramp.tile([2 * H, TL], F16, name=f"bkv{l}",
                                              tag="bk")
                    else:
                        bounce_k = dramp.tile([H, TL], F16, name=f"bk{l}", tag="bk")
                    for of in range(8):
                        pq = ps_sc.tile([128, TL], F32, name=f"pk{l}{of}", tag="sc")
                        for k in range(8):
                            nc.tensor.matmul(pq[:], wkr[k][:, of * 128:(of + 1) * 128],
                                             hT[k][:], start=(k == 0), stop=(k == 7))
                        t = qktp.tile([128, TL], F16, name=f"kt{l}{of}", tag="qkt")
                        nc.vector.tensor_scalar_add(out=t[:], in0=pq[:],
                                                    scalar1=kb_sb[:, of:of + 1])
                        nc.sync.dma_start(out=bounce_k[of * 128:(of + 1) * 128, :],
                                          in_=t[:])
                    if not is_local:
                        gath_k = dramp.tile([NCORES * H, TL], F16, name=f"gk{l}",
                                            tag="gk", addr_space="Shared")
                        nc.gpsimd.collective_compute("AllGather",
                                                     mybir.AluOpType.bypass,
                                                     replica_groups=RG,
                                                     ins=[bounce_k[:]],
                                                     outs=[gath_k[:]])

                    # ---- v (bf16 out; ctx matmul runs bf16 at 1 cyc/row) ----
                    wvr = load_w2(wv_e, l, "wv")
                    if not is_local:
                        bounce_v = dramp.tile([TL, 2 * H], BF16, name=f"bv{l}", tag="bv")
                    for tt in range(2):
                        vt = vsbp.tile([128, 2 * H], BF16, name=f"v{l}{tt}", tag="vsb")
                        for nn in range(2):
                            pv = ps_mm.tile([128, 512], F32, name=f"pv{l}{tt}{nn}",
                                            tag="mm")
                            for k in range(8):
                                nc.tensor.matmul(pv[:], hT[k][:, tt * 128:(tt + 1) * 128],
                                                 wvr[k][:, nn * 512:(nn + 1) * 512],
                                                 start=(k == 0), stop=False)
                            nc.tensor.matmul(pv[:], ones_row16[:, 0:128],
                                             vb_sb[:, nn * 512:(nn + 1) * 512],
                                             start=False, stop=True)
                            # hi/lo bf16 split: v = hi + lo to ~17 bits
                            nc.vector.tensor_copy(out=vt[:, nn * 512:(nn + 1) * 512],
                                                  in_=pv[:])
                            nc.vector.tensor_tensor(
                                out=vt[:, H + nn * 512:H + (nn + 1) * 512],
                                in0=pv[:], in1=vt[:, nn * 512:(nn + 1) * 512],
                                op=mybir.AluOpType.subtract)
                        if is_local:
                            nc.sync.dma_start(
                                out=bounce_k[H + tt * 512:H + (tt + 1) * 512, :]
                                    .bitcast(BF16)
                                    .rearrange("(p a) j -> p a j", p=128),
                                in_=vt[:])
                        else:
                            nc.sync.dma_start(out=bounce_v[tt * 128:(tt + 1) * 128, :],
                                              in_=vt[:])
                    if is_local:
                        # two pair AllGathers: every core ends with [left, own]
                        # k/v in its designated buffer (odd cores: A, even: B)
                        gkvA = dramp.tile([2 * 2 * H, TL], F16, name=f"gkvA{l}",
                                          tag="gk", addr_space="Shared")
                        gkvB = dramp.tile([2 * 2 * H, TL], F16, name=f"gkvB{l}",
                                          tag="gv", addr_space="Shared")
                        nc.gpsimd.collective_compute(
                            "AllGather", mybir.AluOpType.bypass,
                            replica_groups=[[0, 1], [2, 3], [4, 5], [6, 7]],
                            ins=[bounce_k[:]], outs=[gkvA[:]])
                        nc.gpsimd.collective_compute(
                            "AllGather", mybir.AluOpType.bypass,
                            replica_groups=[[0, 7], [1, 2], [3, 4], [5, 6]],
                            ins=[bounce_k[:]], outs=[gkvB[:]])
                    else:
                        gath_v = dramp.tile([T, 2 * H], BF16, name=f"gv{l}", tag="gv",
                                            addr_space="Shared")
                        nc.gpsimd.collective_compute("AllGather",
                                                     mybir.AluOpType.bypass,
                                                     replica_groups=RG,
                                                     ins=[bounce_v[:]],
                                                     outs=[gath_v[:]])

                    # ---- qT (stays local) ----
                    wqr = load_w2(wq_e, l, "wq")
                    qt = []
                    for of in range(8):
                        pq = ps_sc.tile([128, TL], F32, name=f"pq{l}{of}", tag="sc")
                        for k in range(8):
                            nc.tensor.matmul(pq[:], wqr[k][:, of * 128:(of + 1) * 128],
                                             hT[k][:], start=(k == 0), stop=(k == 7))
                        t = qktp.tile([128, TL], F16, name=f"qt{l}{of}", tag="qkt")
                        nc.vector.tensor_scalar_add(out=t[:], in0=pq[:],
                                                    scalar1=qb_sb[:, of:of + 1])
                        qt.append(t)

                    # ---- attention ----
                    if is_local:
                        ctxT = [None] * HP
                        gA3 = gkvA[:].rearrange("(m p) j -> m p j", m=2)
                        gB3 = gkvB[:].rearrange("(m p) j -> m p j", m=2)
                        vloc = []
                        for X3, Xn in ((gA3, "A"), (gB3, "B")):
                            vm = []
                            for m in range(2):
                                vt2 = vgp.tile([128, 2048], BF16,
                                               name=f"vl{l}{Xn}{m}", tag="vg")
                                nc.sync.dma_start(
                                    out=vt2[:],
                                    in_=X3[m, H:2 * H, :].bitcast(BF16)
                                        .rearrange("(u p a) j -> p u a j",
                                                   u=2, p=128))
                                vm.append(vt2)
                            vloc.append(vm)
                        mloc = maskp.tile([128, 8 * TL], BF16, name=f"ml{l}",
                                          tag="mask")
                        nc.sync.dma_start(out=mloc[:],
                                          in_=ml_e[:].rearrange("a p j -> p a j"))
                        for hp in range(HP):
                            kts = []
                            for X3, Xn in ((gA3, "A"), (gB3, "B")):
                                kt = ktgp.tile([128, 512], F16,
                                               name=f"ktl{l}{hp}{Xn}", tag="ktg")
                                nc.sync.dma_start(
                                    out=kt[:],
                                    in_=X3[:, hp * 128:(hp + 1) * 128, :]
                                        .transpose([1, 0, 2]))
                                kts.append(kt)
                            pcs = ps_ctx.tile([128, 512], F32, name=f"pcl{l}{hp}",
                                              tag="ctx")
                            nc.vector.memset(pcs[:], 0.0)
                            for s in range(8):
                                X = s // 4
                                m = (s % 4) // 2
                                u = s % 2
                                ksl = slice(m * 256 + u * 128,
                                            m * 256 + (u + 1) * 128)
                                s0 = ps_sc.tile([128, TL], F32,
                                                name=f"ls0_{l}{hp}{s}", tag="sc")
                                s1 = ps_sc.tile([128, TL], F32,
                                                name=f"ls1_{l}{hp}{s}", tag="sc")
                                nc.tensor.matmul(s0[:], kts[X][0:64, ksl],
                                                 qt[hp][0:64, :],
                                                 start=True, stop=True)
                                nc.tensor.matmul(s1[:], kts[X][64:128, ksl],
                                                 qt[hp][64:128, :],
                                                 start=True, stop=True)
                                ef0 = evp.tile([128, TL], F32,
                                               name=f"lef0_{l}{hp}{s}", tag="ev")
                                ef1 = evp.tile([128, TL], F32,
                                               name=f"lef1_{l}{hp}{s}", tag="ev")
                                msl = mloc[:, s * TL:(s + 1) * TL]
                                nc.vector.tensor_tensor(out=ef0[:], in0=s0[:],
                                                        in1=msl,
                                                        op=mybir.AluOpType.add)
                                nc.vector.tensor_tensor(out=ef1[:], in0=s1[:],
                                                        in1=msl,
                                                        op=mybir.AluOpType.add)
                                e0 = evbp.tile([128, TL], BF16,
                                               name=f"le0_{l}{hp}{s}", tag="evb")
                                e1 = evbp.tile([128, TL], BF16,
                                               name=f"le1_{l}{hp}{s}", tag="evb")
                                nc.scalar.activation(out=e0[:], in_=ef0[:],
                                                     func=mybir.ActivationFunctionType.Exp)
                                nc.scalar.activation(out=e1[:], in_=ef1[:],
                                                     func=mybir.ActivationFunctionType.Exp)
                                sp = (s == 7)
                                vb0 = u * 1024 + hp * 128
                                nc.tensor.matmul(pcs[0:64, 0:TL],
                                                 vloc[X][m][:, vb0:vb0 + 64], e0[:],
                                                 start=False, stop=sp,
                                                 tile_position=(0, 0),
                                                 skip_group_check=True)
                                nc.tensor.matmul(pcs[64:128, 0:TL],
                                                 vloc[X][m][:, vb0 + 64:vb0 + 128],
                                                 e1[:],
                                                 start=False, stop=sp,
                                                 tile_position=(0, 64),
                                                 skip_group_check=True)
                                nc.tensor.matmul(pcs[0:32, TL:2 * TL], ones_col[:],
                                                 e0[:],
                                                 start=False, stop=sp,
                                                 tile_position=(0, 0),
                                                 skip_group_check=True)
                                nc.tensor.matmul(pcs[32:64, TL:2 * TL], ones_col[:],
                                                 e1[:],
                                                 start=False, stop=sp,
                                                 tile_position=(0, 32),
                                                 skip_group_check=True)
                            rsA = smallp.tile([1, TL], F32, name=f"lrsA{l}{hp}",
                                              tag="rsA")
                            rsB = smallp.tile([1, TL], F32, name=f"lrsB{l}{hp}",
                                              tag="rsB")
                            nc.vector.reciprocal(out=rsA[:],
                                                 in_=pcs[0:1, TL:2 * TL])
                            nc.vector.reciprocal(out=rsB[:],
                                                 in_=pcs[32:33, TL:2 * TL])
                            pbc = ps_sc.tile([128, TL], F32, name=f"lpbc{l}{hp}",
                                             tag="sc")
                            nc.tensor.matmul(pbc[0:64, :], ones_row32[:, 0:64],
                                             rsA[:], start=True, stop=True,
                                             tile_position=(0, 0))
                            nc.tensor.matmul(pbc[64:128, :], ones_row32[:, 0:64],
                                             rsB[:], start=True, stop=True,
                                             tile_position=(0, 64))
                            rb = rbp.tile([128, TL], F32, name=f"lrb{l}{hp}",
                                          tag="rb")
                            nc.vector.tensor_copy(out=rb[:], in_=pbc[:])
                            ct = ctxTp.tile([128, TL], F16, name=f"lct{l}{hp}",
                                            tag="ctxT")
                            nc.vector.tensor_tensor(out=ct[:], in0=pcs[:, 0:TL],
                                                    in1=rb[:],
                                                    op=mybir.AluOpType.mult)
                            ctxT[hp] = ct
                        # skip the global-attention path below
                        attn_done = True
                    else:
                        attn_done = False
                    # ---- global attention: kb-half outer, head-pair inner ----
                    if not attn_done:
                     gk3 = gath_k[:].rearrange("(c p) j -> c p j", c=NCORES)
                     acc_t = [None] * HP
                     ctxT = [None] * HP
                     for half in range(2):
                        kbs = range(half * 8, half * 8 + 8)
                        vg4 = []
                        for j in range(4):
                            q0 = half * 1024 + j * 256
                            vt4 = vgp.tile([128, 2 * 2 * H], BF16,
                                           name=f"vg{l}{half}{j}", tag="vg")
                            nc.sync.dma_start(
                                out=vt4[:],
                                in_=gath_v[q0:q0 + 256, :]
                                    .rearrange("(a p) j -> p a j", p=128))
                            vg4.append(vt4)

                        def vgs(kb, cols, lo=False):
                            r = kb - half * 8
                            base = (r % 2) * 2 * H + (H if lo else 0)
                            return vg4[r // 2][:, base + cols.start:base + cols.stop]

                        for hp in range(HP):
                            kt4 = ktgp.tile([128, 4 * TL], F16, name=f"ktg{l}{half}{hp}",
                                            tag="ktg")
                            nc.sync.dma_start(
                                out=kt4[:],
                                in_=gk3[half * 4:(half + 1) * 4,
                                        hp * 128:(hp + 1) * 128, :]
                                    .transpose([1, 0, 2]))

                            pcs = ps_ctx.tile([128, 512], F32, name=f"pcs{l}{half}{hp}",
                                              tag="ctx")
                            # interleaved accumulation groups share this bank; a
                            # start=True would mark the whole 2KB bank row pending-
                            # zero and wipe sibling groups, so init via memset and
                            # accumulate with start=False throughout.
                            nc.vector.memset(pcs[:], 0.0)
                            for kb in kbs:
                                cc, hf = kb // 2, kb % 2
                                ksl = slice((cc - half * 4) * TL + hf * 128,
                                            (cc - half * 4) * TL + (hf + 1) * 128)
                                s0 = ps_sc.tile([128, TL], F32, name=f"s0_{l}{hp}{kb}",
                                                tag="sc")
                                s1 = ps_sc.tile([128, TL], F32, name=f"s1_{l}{hp}{kb}",
                                                tag="sc")
                                nc.tensor.matmul(s0[:], kt4[0:64, ksl], qt[hp][0:64, :],
                                                 start=True, stop=True)
                                nc.tensor.matmul(s1[:], kt4[64:128, ksl],
                                                 qt[hp][64:128, :],
                                                 start=True, stop=True)
                                ef0 = evp.tile([128, TL], F32, name=f"ef0_{l}{hp}{kb}",
                                               tag="ev")
                                ef1 = evp.tile([128, TL], F32, name=f"ef1_{l}{hp}{kb}",
                                               tag="ev")
                                nc.vector.tensor_tensor(out=ef0[:], in0=s0[:],
                                                        in1=mt(kb),
                                                        op=mybir.AluOpType.add)
                                nc.vector.tensor_tensor(out=ef1[:], in0=s1[:],
                                                        in1=mt(kb),
                                                        op=mybir.AluOpType.add)
                                nc.scalar.activation(out=ef0[:], in_=ef0[:],
                                                     func=mybir.ActivationFunctionType.Exp)
                                nc.scalar.activation(out=ef1[:], in_=ef1[:],
                                                     func=mybir.ActivationFunctionType.Exp)
                                eh0 = evbp.tile([128, TL], BF16,
                                                name=f"eh0_{l}{hp}{kb}", tag="evb")
                                el0 = evbp.tile([128, TL], BF16,
                                                name=f"el0_{l}{hp}{kb}", tag="evb")
                                eh1 = evbp.tile([128, TL], BF16,
                                                name=f"eh1_{l}{hp}{kb}", tag="evb")
                                el1 = evbp.tile([128, TL], BF16,
                                                name=f"el1_{l}{hp}{kb}", tag="evb")
                                nc.vector.tensor_copy(out=eh0[:], in_=ef0[:])
                                nc.vector.tensor_tensor(out=el0[:], in0=ef0[:],
                                                        in1=eh0[:],
                                                        op=mybir.AluOpType.subtract)
                                nc.vector.tensor_copy(out=eh1[:], in_=ef1[:])
                                nc.vector.tensor_tensor(out=el1[:], in0=ef1[:],
                                                        in1=eh1[:],
                                                        op=mybir.AluOpType.subtract)
                                sp = (kb == half * 8 + 7)
                                slA = slice(hp * 128, hp * 128 + 64)
                                slB = slice(hp * 128 + 64, (hp + 1) * 128)
                                for (vlo, ee) in ((False, eh0), (False, el0),
                                                  (True, eh0)):
                                    nc.tensor.matmul(pcs[0:64, 0:TL],
                                                     vgs(kb, slA, lo=vlo), ee[:],
                                                     start=False,
                                                     stop=(sp and vlo),
                                                     tile_position=(0, 0),
                                                     skip_group_check=True)
                                for (vlo, ee) in ((False, eh1), (False, el1),
                                                  (True, eh1)):
                                    nc.tensor.matmul(pcs[64:128, 0:TL],
                                                     vgs(kb, slB, lo=vlo), ee[:],
                                                     start=False,
                                                     stop=(sp and vlo),
                                                     tile_position=(0, 64),
                                                     skip_group_check=True)
                                nc.tensor.matmul(pcs[0:32, TL:2 * TL], ones_col[:],
                                                 eh0[:], start=False, stop=False,
                                                 tile_position=(0, 0),
                                                 skip_group_check=True)
                                nc.tensor.matmul(pcs[0:32, TL:2 * TL], ones_col[:],
                                                 el0[:], start=False, stop=sp,
                                                 tile_position=(0, 0),
                                                 skip_group_check=True)
                                nc.tensor.matmul(pcs[32:64, TL:2 * TL], ones_col[:],
                                                 eh1[:], start=False, stop=False,
                                                 tile_position=(0, 32),
                                                 skip_group_check=True)
                                nc.tensor.matmul(pcs[32:64, TL:2 * TL], ones_col[:],
                                                 el1[:], start=False, stop=sp,
                                                 tile_position=(0, 32),
                                                 skip_group_check=True)
                            if half == 0:
                                a = accp.tile([128, 512], F32, name=f"ac{l}{hp}",
                                              tag="acc")
                                nc.vector.tensor_copy(out=a[:], in_=pcs[:])
                                acc_t[hp] = a
                            else:
                                comb = accp.tile([128, 512], F32, name=f"cb{l}{hp}",
                                                 tag="acc")
                                nc.vector.tensor_tensor(out=comb[:], in0=pcs[:],
                                                        in1=acc_t[hp][:],
                                                        op=mybir.AluOpType.add)
                                rsA = smallp.tile([1, TL], F32, name=f"rsA{l}{hp}",
                                                  tag="rsA")
                                rsB = smallp.tile([1, TL], F32, name=f"rsB{l}{hp}",
                                                  tag="rsB")
                                nc.vector.reciprocal(out=rsA[:],
                                                     in_=comb[0:1, TL:2 * TL])
                                nc.vector.reciprocal(out=rsB[:],
                                                     in_=comb[32:33, TL:2 * TL])
                                pbc = ps_sc.tile([128, TL], F32, name=f"pbc{l}{hp}",
                                                 tag="sc")
                                nc.tensor.matmul(pbc[0:64, :], ones_row32[:, 0:64],
                                                 rsA[:],
                                                 start=True, stop=True,
                                                 tile_position=(0, 0))
                                nc.tensor.matmul(pbc[64:128, :], ones_row32[:, 0:64],
                                                 rsB[:],
                                                 start=True, stop=True,
                                                 tile_position=(0, 64))
                                rb = rbp.tile([128, TL], F32, name=f"rb{l}{hp}",
                                              tag="rb")
                                nc.vector.tensor_copy(out=rb[:], in_=pbc[:])
                                ct = ctxTp.tile([128, TL], F16, name=f"ct{l}{hp}",
                                                tag="ctxT")
                                nc.vector.tensor_tensor(out=ct[:], in0=comb[:, 0:TL],
                                                        in1=rb[:],
                                                        op=mybir.AluOpType.mult)
                                ctxT[hp] = ct

                    # ---- attention out projection + residual ----
                    wor = load_w2(wo_e, l, "wo")
                    x_new = []
                    for tt in range(2):
                        xt = xresp.tile([128, H], F32, name=f"xa{l}{tt}", tag="x")
                        for nn in range(2):
                            pa = ps_mm.tile([128, 512], F32, name=f"pa{l}{tt}{nn}",
                                            tag="mm")
                            for k in range(8):
                                nc.tensor.matmul(pa[:], ctxT[k][:, tt * 128:(tt + 1) * 128],
                                                 wor[k][:, nn * 512:(nn + 1) * 512],
                                                 start=(k == 0), stop=False)
                            nc.tensor.matmul(pa[:], ones_row16[:, 0:128],
                                             ob_sb[:, nn * 512:(nn + 1) * 512],
                                             start=False, stop=True)
                            nc.vector.tensor_tensor(out=xt[:, nn * 512:(nn + 1) * 512],
                                                    in0=pa[:],
                                                    in1=x_cur[tt][:, nn * 512:(nn + 1) * 512],
                                                    op=mybir.AluOpType.add)
                        x_new.append(xt)
                    x_cur = x_new

                    # ---- MLP: fc streams of-pairs, proj accumulates across all
                    # 32 k-blocks in 4 live PSUM groups ----
                    h2 = layernorm_f16(x_cur, f"l{l}b")
                    h2T = transpose_h(h2, f"l{l}b")
                    pps = [[None, None], [None, None]]
                    for tt in range(2):
                        for nn in range(2):
                            pool = ps_ctx if tt == 0 else ps_mm
                            pps[tt][nn] = pool.tile([128, 512], F32,
                                                    name=f"pp{l}{tt}{nn}",
                                                    tag="ctx" if tt == 0 else "mm")
                    x_new = [xresp.tile([128, H], F32, name=f"xm{l}{tt}", tag="x")
                             for tt in range(2)]
                    for halfk in range(2):
                        gts = []
                        for ofp in range(8):
                            wf2 = wfp.tile([128, 2048], F16,
                                           name=f"wf{l}{halfk}{ofp}", tag="wf")
                            c0 = halfk * 2048 + ofp * 256
                            nc.gpsimd.dma_start(
                                out=wf2[:],
                                in_=wf_e[l, :, c0:c0 + 256]
                                    .rearrange("(a p) (o j) -> p a o j", p=128, o=2))
                            for o in range(2):
                                of = halfk * 16 + ofp * 2 + o
                                pf = ps_sc.tile([128, TL], F32, name=f"pf{l}{of}",
                                                tag="sc")
                                for k in range(8):
                                    nc.tensor.matmul(
                                        pf[:],
                                        wf2[:, k * 256 + o * 128:k * 256 + o * 128 + 128],
                                        h2T[k][:],
                                        start=(k == 0), stop=(k == 7))
                                g = gtp.tile([128, TL], F16, name=f"g{l}{of}", tag="g")
                                nc.scalar.activation(out=g[:], in_=pf[:],
                                                     func=mybir.ActivationFunctionType.Gelu,
                                                     bias=fb_sb[:, of:of + 1], scale=1.0)
                                gts.append(g)
                        for kk2 in range(8):
                            w2 = wprp.tile([128, 2048], F16,
                                           name=f"wp{l}{halfk}{kk2}", tag="wpr")
                            r0 = halfk * 2048 + kk2 * 256
                            nc.gpsimd.dma_start(
                                out=w2[:],
                                in_=wp_e[l, r0:r0 + 256, :]
                                    .rearrange("(a p) j -> p a j", p=128))
                            for a in range(2):
                                kk = kk2 * 2 + a
                                for tt in range(2):
                                    for nn in range(2):
                                        nc.tensor.matmul(
                                            pps[tt][nn][:],
                                            gts[kk][:, tt * 128:(tt + 1) * 128],
                                            w2[:, a * H + nn * 512:a * H + (nn + 1) * 512],
                                            start=(halfk == 0 and kk == 0),
                                            stop=False)
                    for tt in range(2):
                        for nn in range(2):
                            nc.tensor.matmul(pps[tt][nn][:], ones_row16[:, 0:128],
                                             pb_sb[:, nn * 512:(nn + 1) * 512],
                                             start=False, stop=True)
                            nc.vector.tensor_tensor(
                                out=x_new[tt][:, nn * 512:(nn + 1) * 512],
                                in0=pps[tt][nn][:],
                                in1=x_cur[tt][:, nn * 512:(nn + 1) * 512],
                                op=mybir.AluOpType.add)
                    x_cur = x_new

                if not with_logits:
                    for tt in range(2):
                        nc.sync.dma_start(out=out_e[tt * 128:(tt + 1) * 128, :],
                                          in_=x_cur[tt][:])
                    gath_x = None
                else:
                    xh = layernorm_f16(x_cur, "f")
                    xhT = transpose_h(xh, "f")
                    bounce_x = dramp.tile([H, TL], F16, name="bx", tag="bx")
                    for hk in range(8):
                        nc.sync.dma_start(out=bounce_x[hk * 128:(hk + 1) * 128, :],
                                          in_=xhT[hk][:])
                    gath_x = dramp.tile([NCORES * H, TL], F16, name="gx", tag="gx",
                                        addr_space="Shared")
                    nc.gpsimd.collective_compute("AllGather", mybir.AluOpType.bypass,
                                                 replica_groups=RG,
                                                 ins=[bounce_x[:]], outs=[gath_x[:]])

            if with_logits:
                with ExitStack() as _gstk:
                    _gp = lambda *a, **kw: _gstk.enter_context(tc.tile_pool(*a, **kw))
                    xtgp = _gp(name="xtg", bufs=8)   # [128,2048] f16 resident xT
                    lmp = _gp(name="lmp", bufs=4)    # [128,2048] f16 lm vv-pairs
                    outp = _gp(name="outp", bufs=3)  # [128,2048] f16 out rows

                    lbt_sb = smallp.tile([128, VSH // 128], F32, name="lbt", tag="lbt")
                    nc.sync.dma_start(out=lbt_sb[:], in_=lbt_e[:])
                    gx3 = gath_x[:].rearrange("(c p) j -> c p j", c=NCORES)
                    # all 8 xT tiles stay resident so lm streams exactly once;
                    # xtg[k][:, cc*256:(cc+1)*256] = gath_x core cc, k-block k
                    xtg = []
                    for k in range(8):
                        t = xtgp.tile([128, 2048], F16, name=f"xtg{k}", tag="xtg")
                        nc.sync.dma_start(
                            out=t[:],
                            in_=gx3[:, k * 128:(k + 1) * 128, :].transpose([1, 0, 2]))
                        xtg.append(t)
                    for vp in range(VSH // 256):
                        lm2 = lmp.tile([128, 2048], F16, name=f"lm{vp}", tag="lmp")
                        nc.gpsimd.dma_start(
                            out=lm2[:],
                            in_=lm_e[:, vp * 256:(vp + 1) * 256]
                                .rearrange("(a p) (o j) -> p a o j", p=128, o=2))
                        for o in range(2):
                            vv = vp * 2 + o
                            ot = outp.tile([128, 2048], F16, name=f"o{vv}", tag="outp")
                            for tc4 in range(4):
                                pl = ps_mm.tile([128, 512], F32, name=f"pl{tc4}{vv}",
                                                tag="mm")
                                for k in range(8):
                                    nc.tensor.matmul(
                                        pl[:],
                                        lm2[:, k * 256 + o * 128:k * 256 + o * 128 + 128],
                                        xtg[k][:, tc4 * 512:(tc4 + 1) * 512],
                                        start=(k == 0), stop=(k == 7))
                                nc.vector.tensor_scalar_add(
                                    out=ot[:, tc4 * 512:(tc4 + 1) * 512], in0=pl[:],
                                    scalar1=lbt_sb[:, vv:vv + 1])
                            nc.sync.dma_start(out=out_e[vv * 128:(vv + 1) * 128, :],
                                              in_=ot[:])

    nc.finalize()
    return nc


# ------------------- host-side prep & entry -------------------

def _prep_inputs(inputs, n_layers=NL, with_logits=True):
    f32 = np.float32
    f16 = np.float16
    import ml_dtypes
    bf16 = ml_dtypes.bfloat16

    ids = np.asarray(inputs["input_ids"]).reshape(-1).astype(np.int64)
    wte = np.asarray(inputs["wte"], f32)
    wpe = np.asarray(inputs["wpe"], f32)
    x0 = wte[ids] + wpe[:T]

    wq = np.empty((n_layers, H, H), f16); wk = np.empty((n_layers, H, H), f16)
    wv = np.empty((n_layers, H, H), f16); wo = np.empty((n_layers, H, H), f16)
    wf = np.empty((n_layers, H, MLP), f16); wp = np.empty((n_layers, MLP, H), f16)
    qb = np.empty((n_layers, 128, 8), f32); kbb = np.empty((n_layers, 128, 8), f32)
    vb = np.empty((n_layers, 1, H), f16); ob = np.empty((n_layers, 1, H), f16)
    fb = np.empty((n_layers, 128, 32), f32); pb = np.empty((n_layers, 1, H), f16)
    for l in range(n_layers):
        ln1w = np.asarray(inputs["ln1_w"][l], f32); ln1b = np.asarray(inputs["ln1_b"][l], f32)
        ln2w = np.asarray(inputs["ln2_w"][l], f32); ln2b = np.asarray(inputs["ln2_b"][l], f32)
        for (wdst, bdst, wname) in ((wq, qb, "q_w"), (wk, kbb, "k_w")):
            w = np.asarray(inputs[wname][l], f32)
            wdst[l] = (ln1w[:, None] * w).astype(f16)
            bdst[l] = (ln1b @ w).reshape(8, 128).T
        w = np.asarray(inputs["v_w"][l], f32)
        wv[l] = (ln1w[:, None] * w).astype(f16)
        vb[l] = (ln1b @ w)[None, :].astype(f16)
        wo[l] = np.asarray(inputs["o_w"][l], f32).astype(f16)
        ob[l] = np.asarray(inputs["o_b"][l], f32)[None, :].astype(f16)
        w = np.asarray(inputs["fc_w"][l], f32)
        wf[l] = (ln2w[:, None] * w).astype(f16)
        fbv = np.asarray(inputs["fc_b"][l], f32) + ln2b @ w
        fb[l] = fbv.reshape(32, 128).T
        wp[l] = np.asarray(inputs["proj_w"][l], f32).astype(f16)
        pb[l] = np.asarray(inputs["proj_b"][l], f32)[None, :].astype(f16)

    lnfw = np.asarray(inputs["lnf_w"], f32); lnfb = np.asarray(inputs["lnf_b"], f32)
    VP = NCORES * VSH
    lm_pad = np.zeros((VP, H), f16)
    lm_pad[:VOCAB] = (wte * lnfw[None, :]).astype(f16)
    lb_pad = np.zeros((VP,), f32)
    lb_pad[:VOCAB] = wte @ lnfb

    in_maps = []
    for c in range(NCORES):
        ts = c * TL
        qi = ts + np.arange(TL)[None, :]
        kj = np.arange(128)[:, None]
        mg = np.empty((KB, 128, TL), bf16)
        mlm = np.empty((KB, 128, TL), bf16)
        for kb in range(KB):
            ka = kb * 128 + kj
            causal = (ka <= qi)
            mg[kb] = np.where(causal, 0.0, -30000.0).astype(bf16)
            mlm[kb] = np.where(causal & (qi - ka < WINDOW), 0.0, -30000.0).astype(bf16)
        m = {
            "x0": np.ascontiguousarray(x0[ts:ts + TL]).astype(f32),
            "wq": wq, "wk": wk, "wv": wv, "wo": wo, "wf": wf, "wp": wp,
            "qb": qb, "kb": kbb, "vb": vb, "ob": ob, "fb": fb, "pb": pb,
            "maskg": mg, "maskl": mlm,
        }
        if with_logits:
            m["lm"] = np.ascontiguousarray(lm_pad[c * VSH:(c + 1) * VSH].T)
            m["lbt"] = np.ascontiguousarray(
                lb_pad[c * VSH:(c + 1) * VSH].reshape(VSH // 128, 128).T)
        in_maps.append(m)
    return in_maps


_NC_CACHE = {}


def _get_nc(n_layers=NL, with_logits=True):
    key = (n_layers, with_logits)
    if key not in _NC_CACHE:
        _NC_CACHE[key] = build(n_layers, with_logits)
    return _NC_CACHE[key]


def run(inputs, n_layers=NL, with_logits=True, trace=False):
    nc = _get_nc(n_layers, with_logits)
    in_maps = _prep_inputs(inputs, n_layers, with_logits)
    res = run_bass_kernel_spmd(nc, in_maps, list(range(NCORES)), trace=trace)
    if with_logits:
        parts = [res.results[c]["out"] for c in range(NCORES)]   # each [VSH, T] f16
        full = np.concatenate(parts, axis=0)[:VOCAB]             # [VOCAB, T]
        out = np.ascontiguousarray(full.T.astype(np.float32))[None]  # [1, T, VOCAB]
    else:
        out = np.concatenate([res.results[c]["out"] for c in range(NCORES)], axis=0)[None]
    return out, res


def kernel(**inputs) -> np.ndarray:
    out, _ = run(inputs, NL, True, trace=False)
    return out
